# revision 1
# baseline (speedup 1.0000x reference)
"""Trainium2 Bass kernel for nn_Co_Pam_Module (PAM-style sparse attention +
nearest-upsample + BatchNorm residual).

Sharding: data-parallel over batch B=8 across 8 NeuronCores (one batch per
core); BN batch statistics are synchronized with a tiny AllReduce.

Math (validated vs reference, rel err ~1e-6 in numpy):
  q = wq@y + bq            [32, 2048]
  k = wk@y + bk            [32, 2048]
  E^T[t,s] = sum_d k[d,t] q[d,s]        (energy transposed; range ~±31 so
  P^T = exp(E^T)                         no max-subtraction is needed in f32)
  x_pool[c,j] = sum_u x[c,4j+u]
  vmm = (gamma*wv) @ x_pool             (gamma folded into weights)
  O~g[c,i] = sum_t vmm^T[t,c]*P^T[t,i]  via matmul with vpT=[vmm^T | ones];
  s[i]    = column 64 of the same accumulation (softmax denominator)
  G = O~g/s + 4*gamma*bv ; sync-BN stats via AllReduce of (sum G, sum G^2)
  out = x + scale_c*G_rep4 + bias_c
"""

import numpy as np

import concourse.bass as bass
import concourse.tile as tile
from concourse import mybir
from concourse.vector_clock import ScopedClock

F32 = mybir.dt.float32
F32R = mybir.dt.float32r
AF = mybir.ActivationFunctionType
ALU = mybir.AluOpType

SYNC_MODE = "collective"  # "collective" | "rdma" | "none"

B, CX, HX, WX = 8, 64, 128, 64
CY, HY, WY = 256, 64, 32
SX, SY, D, RUP = HX * WX, HY * WY, 32, 4  # 8192, 2048, 32, 4
N_CORES = 8
BN_EPS = 1e-5
WPK_COLS = 840


# ---------------------------------------------------------------------------
# Workaround: walrus in this container rejects >cap sem waits on the Tile
# kernel-tail Drain.  Emit explicit per-sem wait_ge instructions instead.
def _patched_drain_and_barrier(self, tick_clock, wait_clock):
    nc = self.nc
    probe = nc.sync.nop(nofuse=True)
    wait_clock.add_sem_waits(probe.ins, ScopedClock({None: tick_clock.global_clock}))
    waits = list(probe.ins.sync_info.on_wait)
    probe.ins.sync_info.on_wait = []
    name2handle = {}
    for k, h in wait_clock.sems.allocated().items():
        name2handle[getattr(h, "name", str(k))] = h
    for w in waits:
        h = name2handle.get(w.ant_name)
        if h is None:
            raise RuntimeError(f"no sem handle for {w.ant_name}")
        nc.sync.wait_ge(h, w.wait_value)
    nc.sync.drain()
    nc.all_engine_barrier()
    popped = nc._tile_sem_poison_stack.pop()
    assert popped is self._sem_poison
    nc.clear_and_free_semaphores(list(self.sems.allocated().values()))
    nc.all_engine_barrier()


tile.TileContext._drain_and_barrier = _patched_drain_and_barrier


def _split_excess_waits(nc, cap=1):
    """Walrus in this container allows only `cap` sem waits per instruction.
    Hoist excess semaphore waits onto same-engine NoOps inserted just before
    the instruction (same engine + program order => semantics preserved)."""
    n_split = 0
    for f in nc.m.functions:
        for blk in f.blocks:
            insts = list(blk.instructions)
            new_insts = []
            changed = False
            for inst in insts:
                si = inst.sync_info
                waits = list(si.on_wait) if si is not None else []
                if len(waits) > cap:
                    sem_w = [w for w in waits if w.sync_type == "semaphore"]
                    other_w = [w for w in waits if w.sync_type != "semaphore"]
                    budget = max(0, cap - len(other_w))
                    keep, excess = sem_w[:budget], sem_w[budget:]
                    for i in range(0, len(excess), max(1, cap)):
                        chunk = excess[i : i + max(1, cap)]
                        nop = mybir.InstNoOp(
                            name=f"{inst.name}-ws{n_split}",
                            sync_info=mybir.SyncInfo(on_wait=chunk, on_update=[]),
                            bass_nofuse=True,
                            engine=inst.engine,
                        )
                        new_insts.append(nop)
                        n_split += 1
                    si.on_wait = other_w + keep
                    changed = True
                new_insts.append(inst)
            if changed:
                blk.instructions = new_insts
    return n_split
# ---------------------------------------------------------------------------


def _rep_ap(ap, rep):
    """Append a step-0 (repeat) innermost free dim to an AP."""
    return bass.AP(tensor=ap.tensor, offset=ap.offset, ap=list(ap.ap) + [[0, rep]])


def build_module(split_waits=True):
    nc = bass.Bass()

    xb = nc.dram_tensor("xb", [128, SX // 2], F32, kind="ExternalInput")
    yb = nc.dram_tensor("yb", [2, 128, SY], F32R, kind="ExternalInput")
    # packed small weights, one DMA: see _host_inputs for the column map
    wpk = nc.dram_tensor("wpk", [128, WPK_COLS], F32R, kind="ExternalInput")
    msc = nc.dram_tensor("msc", [128, 8], F32, kind="ExternalInput")
    out = nc.dram_tensor("out", [128, SX // 2], F32, kind="ExternalOutput")

    with tile.TileContext(nc, num_cores=N_CORES) as tc:
        with (
            tc.tile_pool(name="const", bufs=1) as cp,
            tc.tile_pool(name="big", bufs=1) as big,
            tc.tile_pool(name="ptile", bufs=6) as pp,
            tc.tile_pool(name="dram", bufs=1, space="DRAM") as dp,
        ):
            # ---------------- constants / weights (single DMA) ----------------
            wpk_sb = cp.tile([128, WPK_COLS], F32R)
            nc.sync.dma_start(wpk_sb[:, 0:768], wpk[:, 0:768])
            # column map (f32 cols): 0:256 wqT(kc0,kc1), 256:512 wkT,
            # 512:640 bq row, 640:768 bk row, 768:832 wvT*gamma (stacked
            # twice on partitions); first DMA piece carries everything the
            # q/k matmuls need
            bq_sb = wpk_sb[0:1, 512:640]
            bk_sb = wpk_sb[0:1, 640:768]
            wv_sb = wpk_sb[:, 768:832]
            msc_sb = cp.tile([128, 8], F32)
            bv4g_sb = msc_sb[0:64, 0:1]
            bv4g_sb2 = msc_sb[:, 0:1]  # [128,1]
            c_s1_sb = msc_sb[0:64, 1:2]
            bv4g2_sb = msc_sb[0:64, 2:3]
            c_s2_sb = msc_sb[0:64, 3:4]
            bnw_sb = msc_sb[:, 4:5]  # [128,1] both halves
            bnb_sb = msc_sb[:, 5:6]  # [128,1] both halves

            ones_row = cp.tile([1, 512], F32R)
            nc.vector.memset(ones_row[:].bitcast(F32), 1.0)
            ones64 = cp.tile([1, 64], F32R)
            nc.vector.memset(ones64[:].bitcast(F32), 1.0)
            eps_sb = cp.tile([128, 1], F32)
            nc.vector.memset(eps_sb[:], BN_EPS)

            # prewarm exp table early (overlaps initial DMA)
            warm = cp.tile([1, 8], F32)
            nc.vector.memset(warm[:], 0.0)
            nc.scalar.activation(warm[:], warm[:], AF.Exp)
            # prewarm the PE clock (HAM ramp): dummy matmul chain on a zero tile
            pewarm = cp.tile([128, 512], F32R)
            nc.vector.memset(pewarm[:].bitcast(F32), 0.0)

            # ---------------- big inputs ----------------
            y_sb = big.tile([128, 2, SY], F32R)
            # x in split layout: partition h*64+c holds x[c, 4096h:4096(h+1)]
            x2 = big.tile([128, SX // 2], F32)
            NXP = 4  # x pieces; pooling/vpT chunked to chase the DMA
            def y_quarter(ch):
                for kc in range(2):
                    nc.sync.dma_start(
                        y_sb[:, kc, ch * 512 : (ch + 1) * 512],
                        yb[kc][:, ch * 512 : (ch + 1) * 512],
                    )

            def x_piece(p):
                xsl = slice(p * 1024, (p + 1) * 1024)
                nc.sync.dma_start(x2[:, xsl], xb[:, xsl])

            # ordered so each consumer's operand lands just before its first
            # use: x pieces chase the vpT chain (iter 2p), late y quarters
            # only gate E at iters 8/12
            y_quarter(0)
            y_quarter(1)
            nc.sync.dma_start(wpk_sb[:, 768:], wpk[:, 768:])
            x_piece(0)
            x_piece(1)
            y_quarter(2)
            x_piece(2)
            x_piece(3)
            y_quarter(3)
            nc.sync.dma_start(msc_sb[:], msc[:])

            q_sb = big.tile([128, SY], F32R)
            k_sb = big.tile([128, SY], F32R)

            # ---------------- main compute: single PSUM regime ----------------
            # psE: 3 rotating [128,1024] slots (6 banks) shared by warmup/qk/
            # vpT/rs-broadcast/E tiles; psO: [65,1024] accumulator (2 banks).
            t1 = big.tile([128, SX // 4], F32)
            xp = big.tile([128, SX // 8], F32R)
            xv = x2[:].rearrange("p (n u) -> p n u", u=2)
            tv = t1[:].rearrange("p (n u) -> p n u", u=2)
            vpT = big.tile([128, 16, 65], F32R)
            nc.vector.memset(vpT[:, :, 64:65].bitcast(F32), 1.0)
            G0 = big.tile([64, SY], F32)
            s1_h = cp.tile([64, 2], F32)
            s2_h = cp.tile([64, 2], F32)
            junk2 = big.tile([64, 1024], F32, tag="junk2")
            with (
                tc.tile_pool(name="psE", bufs=3, space="PSUM") as psE,
                tc.tile_pool(name="psO", bufs=1, space="PSUM") as psO,
            ):
                # PE clock warmup
                wslot = psE.tile([128, 1024], F32, tag="E")
                for _ in range(4):
                    nc.tensor.matmul(
                        wslot[:, 0:512], pewarm[:, 0:128], pewarm[:],
                        start=True, stop=True,
                    )

                def emit_qk(w_off, b_t, dst, qt):
                    gslc = slice(qt * 512, (qt + 1) * 512)
                    ps = psE.tile([128, 1024], F32, tag="E")
                    for kc in range(2):
                        nc.tensor.matmul(
                            ps[:, 0:512],
                            wpk_sb[:, w_off + kc * 128 : w_off + kc * 128 + 128],
                            y_sb[:, kc, gslc],
                            start=(kc == 0),
                            stop=False,
                        )
                    nc.tensor.matmul(
                        ps[:, 0:512], b_t[:], ones_row[:],
                        start=False, stop=True,
                    )
                    nc.vector.tensor_copy(dst[:, gslc], ps[:, 0:512])

                emit_qk(0, bq_sb, q_sb, 0)
                emit_qk(256, bk_sb, k_sb, 0)
                emit_qk(0, bq_sb, q_sb, 1)

                def emit_pool_sub(jc):
                    # one 128-col xp window -> vpT chunks {2jc, 2jc+1}; spreads
                    # the piece work over two iterations to balance PE vs ACT
                    t1s = slice(jc * 256, (jc + 1) * 256)
                    nc.vector.tensor_add(t1[:, t1s], xv[:, t1s, 0], xv[:, t1s, 1])
                    xps = slice(jc * 128, (jc + 1) * 128)
                    nc.vector.tensor_add(xp[:, xps], tv[:, xps, 0], tv[:, xps, 1])
                    vps = psE.tile([128, 1024], F32, tag="E")
                    for hh in range(2):
                        base = slice(hh * 64, hh * 64 + 64)
                        # separate banks (cols 0 / 512): start=True bank clears
                        # cannot collide
                        nc.tensor.matmul(
                            vps[0:128, hh * 512 : hh * 512 + 64],
                            xp[base, xps], wv_sb[base, :],
                            start=True, stop=True,
                        )
                    vv = vps[:].rearrange("p (g c) -> p g c", c=512)
                    nc.vector.tensor_copy(vpT[:, 2 * jc : 2 * jc + 2, 0:64], vv[:, :, 0:64])

                def emit_half_tail(h, o_ps):
                    isl_g = slice(h * 1024, (h + 1) * 1024)
                    rs_sb = big.tile([1, 1024], F32R, tag="rs")
                    with nc.allow_low_precision(reason="fp32r softmax denominators"):
                        nc.vector.reciprocal(rs_sb[:], o_ps[64:65, :])
                    rr_ps = psE.tile([128, 1024], F32, tag="E")
                    for c2 in range(2):
                        nc.tensor.matmul(
                            rr_ps[0:64, c2 * 512 : (c2 + 1) * 512],
                            ones64[:],
                            rs_sb[:, c2 * 512 : (c2 + 1) * 512],
                            start=True, stop=True,
                        )
                    rr_sb = big.tile([64, 1024], F32, tag="rsrep_sb")
                    nc.vector.tensor_copy(rr_sb[:], rr_ps[0:64, :])
                    nc.vector.tensor_mul(G0[:, isl_g], o_ps[0:64, :], rr_sb[:])
                    nc.vector.tensor_reduce(
                        s1_h[:, h : h + 1], G0[:, isl_g],
                        mybir.AxisListType.X, ALU.add,
                    )

                def emit_half_sq(h):
                    isl_g = slice(h * 1024, (h + 1) * 1024)
                    nc.scalar.activation(
                        junk2[:], G0[:, isl_g], AF.Square,
                        accum_out=s2_h[:, h : h + 1],
                    )

                o_ps_prev = None
                for h in range(2):
                    o_ps = psO.tile([65, 1024], F32, tag="O")
                    for ts in range(16):
                        tslc = slice(ts * 128, (ts + 1) * 128)
                        e_ps = psE.tile([128, 1024], F32, tag="E")
                        for c2 in range(2):
                            nc.tensor.matmul(
                                e_ps[:, c2 * 512 : (c2 + 1) * 512],
                                k_sb[0:32, tslc],
                                q_sb[0:32, h * 1024 + c2 * 512 : h * 1024 + c2 * 512 + 512],
                                start=True, stop=True,
                            )
                        p_sb = pp.tile([128, 1024], F32R, tag="P")
                        nc.scalar.activation(p_sb[:], e_ps[:], AF.Exp)
                        if h == 0:
                            if ts < 8:
                                emit_pool_sub(ts)
                            if ts == 1:
                                emit_qk(256, bk_sb, k_sb, 1)
                            elif ts == 5:
                                emit_qk(256, bk_sb, k_sb, 2)
                            elif ts == 9:
                                emit_qk(256, bk_sb, k_sb, 3)
                            elif ts == 10:
                                emit_qk(0, bq_sb, q_sb, 2)
                            elif ts == 12:
                                emit_qk(0, bq_sb, q_sb, 3)

                        for c2 in range(2):
                            nc.tensor.matmul(
                                o_ps[:, c2 * 512 : (c2 + 1) * 512],
                                vpT[:, 2 * (ts % 8) + ts // 8, :],
                                p_sb[:, c2 * 512 : (c2 + 1) * 512],
                                start=(ts == 0),
                                stop=(ts == 15),
                                skip_group_check=True,
                            )
                        if h == 1 and ts == 1 and o_ps_prev is not None:
                            emit_half_tail(0, o_ps_prev)
                    o_ps_prev = o_ps
                emit_half_tail(1, o_ps_prev)
                emit_half_sq(0)
                emit_half_sq(1)

            # ---------------- BN stats + AllReduce ----------------
            s1_0 = cp.tile([64, 1], F32)
            s2_0 = cp.tile([64, 1], F32)
            nc.vector.tensor_add(s1_0[:], s1_h[:, 0:1], s1_h[:, 1:2])
            nc.vector.tensor_add(s2_0[:], s2_h[:, 0:1], s2_h[:, 1:2])
            ar_sb = cp.tile([64, 2], F32)
            # s1 = s1_0 + 2048*bv4g
            nc.vector.tensor_add(ar_sb[:, 0:1], s1_0[:], c_s1_sb[:])
            # s2 = s2_0 + 2*bv4g*s1_0 + 2048*bv4g^2
            t2 = cp.tile([64, 1], F32)
            nc.vector.tensor_scalar(
                t2[:], s1_0[:], bv4g2_sb[:], c_s2_sb[:], ALU.mult, ALU.add
            )
            nc.vector.tensor_add(ar_sb[:, 1:2], s2_0[:], t2[:])

            # pre-move G0-hi to partitions 64:128 (hidden under the AllGather)
            G2hi = big.tile([128, 1024], F32, tag="g2hi")
            nc.sync.dma_start(G2hi[64:128, :], G0[:, 1024:2048])

            if SYNC_MODE == "collective":
                sums_tile = cp.tile([128, 2], F32)
                ar_in = dp.tile([64, 2], F32)
                ar_out = dp.tile([N_CORES, 64, 2], F32)
                nc.sync.dma_start(ar_in[:], ar_sb[:])
                nc.gpsimd.collective_compute(
                    "AllGather",
                    ALU.bypass,
                    ins=[ar_in.opt()],
                    outs=[ar_out.opt()],
                    replica_groups=[list(range(N_CORES))],
                )
                # load gathered as [128(dup), 2, 8] (both halves) + rank-reduce
                gath_sb = cp.tile([128, 2, N_CORES], F32)
                for hh in range(2):
                    nc.sync.dma_start(
                        gath_sb[hh * 64 : hh * 64 + 64, :, :],
                        ar_out[:].rearrange("r c j -> c j r"),
                    )
                nc.vector.tensor_reduce(
                    sums_tile[:], gath_sb[:], mybir.AxisListType.X, ALU.add
                )
                sums_sb = sums_tile[:]
            elif SYNC_MODE == "rdma":
                # recursive-doubling allreduce over same-chip peers: 3 rounds of
                # XOR-relative remote DMA broadcasts (single real dest each).
                acc = cp.tile([128, 2], F32)
                nc.vector.memset(acc[:], 0.0)
                nc.vector.tensor_copy(acc[0:64, :], ar_sb[:])
                recvs = cp.tile([128, 3, 2], F32)
                rsems = [nc.alloc_semaphore(f"rdma_r{r}") for r in range(3)]
                lsem = nc.alloc_semaphore("rdma_l")
                with tc.tile_critical():
                    g = nc.gpsimd
                    for r in range(3):
                        delta = 1 << r
                        slot = 4 if (delta & 4) else 0
                        rd = [None] * 8
                        rd[slot] = (0, delta)
                        g.remote_dma_broadcast(
                            recvs[:, r, :],
                            acc[:],
                            remote_sem=rsems[r],
                            local_sem=lsem,
                            rdests=rd,
                        )
                        g.trigger_dma(count=None)
                        g.wait_ge(lsem, 16 * (r + 1))
                        g.wait_ge(rsems[r], 2)
                        g.tensor_add(acc[:], acc[:], recvs[:, r, :])
                sums_sb = acc[0:64, :]
            else:
                # debug fallback: per-core stats scaled by B (exact only if all
                # batches had identical stats)
                sums_tile = cp.tile([128, 2], F32)
                bounce = dp.tile([64, 2], F32)
                nc.sync.dma_start(bounce[:], ar_sb[:])
                for hh in range(2):
                    nc.sync.dma_start(
                        sums_tile[hh * 64 : hh * 64 + 64, :], bounce[:]
                    )
                nc.vector.tensor_scalar_mul(sums_tile[:], sums_tile[:], float(B))
                sums_sb = sums_tile[:]

            # ---------------- scale/bias + final combine ----------------
            sq_warm = cp.tile([64, 1], F32)
            nc.scalar.activation(sq_warm[:], s2_0[:], AF.Sqrt, bias=eps_sb[0:64, :])
            mm_sb = cp.tile([128, 2], F32)
            nc.vector.tensor_scalar_mul(mm_sb[:], sums_sb, float(RUP) / (B * SX))
            m_ap = mm_sb[:, 0:1]
            msq_ap = mm_sb[:, 1:2]
            var_sb = cp.tile([128, 1], F32)
            m2_sb = cp.tile([128, 1], F32)
            nc.vector.tensor_mul(m2_sb[:], m_ap, m_ap)
            nc.vector.tensor_sub(var_sb[:], msq_ap, m2_sb[:])
            std_sb = cp.tile([128, 1], F32)
            nc.scalar.activation(std_sb[:], var_sb[:], AF.Sqrt, bias=eps_sb[:])
            rstd_sb = cp.tile([128, 1], F32)
            nc.vector.reciprocal(rstd_sb[:], std_sb[:])
            scale_sb = cp.tile([128, 1], F32)
            nc.vector.tensor_mul(scale_sb[:], rstd_sb[:], bnw_sb[:])
            # bias2 = bnb - m*scale + scale*bv4g   (both partition halves)
            tb = cp.tile([128, 1], F32)
            nc.vector.tensor_mul(tb[:], m_ap, scale_sb[:])
            bias2_sb = cp.tile([128, 1], F32)
            nc.vector.tensor_sub(bias2_sb[:], bnb_sb[:], tb[:])
            tb2 = cp.tile([128, 1], F32)
            nc.vector.tensor_mul(tb2[:], scale_sb[:], bv4g_sb2)
            nc.vector.tensor_add(bias2_sb[:], bias2_sb[:], tb2[:])

            # R2 split layout [128, 1024]: R2[h*64+c, i'] = scale*G0[c, 1024h+i'] + bias2
            # G0-hi is pre-moved to partitions 64:128 during the AllGather (hidden)
            R2 = big.tile([128, 1024], F32)
            nc.vector.tensor_scalar(
                R2[64:128, :], G2hi[64:128, :], scale_sb[64:128, :],
                bias2_sb[64:128, :], ALU.mult, ALU.add,
            )
            nc.vector.tensor_scalar(
                R2[0:64, :], G0[:, 0:1024], scale_sb[0:64, :],
                bias2_sb[0:64, :], ALU.mult, ALU.add,
            )

            # out2[p, f] = x2[p, f] + R2[p, f>>2]
            out2 = big.tile([128, SX // 2], F32)
            o_view = out2[:].rearrange("p (n u) -> p n u", u=4)
            x_view = x2[:].rearrange("p (n u) -> p n u", u=4)
            NFC = 8
            csz = (SX // 2) // NFC  # 512 output cols -> 128 R cols per chunk
            for qc in range(NFC):
                nsl = slice(qc * (csz // 4), (qc + 1) * (csz // 4))
                nc.vector.tensor_add(
                    o_view[:, nsl, :],
                    x_view[:, nsl, :],
                    _rep_ap(R2[:, nsl], 4),
                )
                nc.sync.dma_start(
                    out[:, qc * csz : (qc + 1) * csz],
                    out2[:, qc * csz : (qc + 1) * csz],
                )

    if split_waits:
        _split_excess_waits(nc)
    return nc


def _host_inputs(x, y, wq, bq, wk, bk, wv, bv, gamma, bn_w, bn_b):
    g = float(np.asarray(gamma).reshape(-1)[0])
    wqT_rep = np.tile(np.ascontiguousarray(wq.T), (1, 4))  # [256, 128]
    wkT_rep = np.tile(np.ascontiguousarray(wk.T), (1, 4))
    bv4g = (4.0 * g * bv)
    wpk = np.zeros((128, WPK_COLS), np.float32)
    wpk[:, 0:128] = wqT_rep[0:128]
    wpk[:, 128:256] = wqT_rep[128:256]
    wpk[:, 256:384] = wkT_rep[0:128]
    wpk[:, 384:512] = wkT_rep[128:256]
    wpk[0, 512:640] = np.tile(bq, 4)
    wpk[0, 640:768] = np.tile(bk, 4)
    wpk[0:64, 768:832] = (g * wv).T
    wpk[64:128, 768:832] = (g * wv).T
    msc = np.zeros((128, 8), np.float32)
    for hh in range(2):
        msc[hh * 64 : hh * 64 + 64, 0] = bv4g
        msc[hh * 64 : hh * 64 + 64, 1] = SY * bv4g
        msc[hh * 64 : hh * 64 + 64, 2] = 2.0 * bv4g
        msc[hh * 64 : hh * 64 + 64, 3] = SY * bv4g * bv4g
        msc[hh * 64 : hh * 64 + 64, 4] = bn_w
        msc[hh * 64 : hh * 64 + 64, 5] = bn_b
    common = {"wpk": wpk, "msc": msc}
    in_maps = []
    for b in range(B):
        m = dict(common)
        # split layout: [2, 64, 4096] where [h, c, f] = x[b, c, 4096h + f]
        xf = np.asarray(x[b], np.float32).reshape(64, 2, SX // 2).transpose(1, 0, 2)
        m["xb"] = np.ascontiguousarray(xf.reshape(128, SX // 2))
        m["yb"] = np.ascontiguousarray(
            np.asarray(y[b], np.float32).reshape(2, 128, SY)
        )
        in_maps.append(m)
    return in_maps


_NC_CACHE = {}


def kernel(x, y, wq, bq, wk, bk, wv, bv, gamma, bn_w, bn_b, _trace=False):
    from concourse.bass_utils import run_bass_kernel_spmd

    if "nc" not in _NC_CACHE:
        _NC_CACHE["nc"] = build_module()
    nc = _NC_CACHE["nc"]
    in_maps = _host_inputs(x, y, wq, bq, wk, bk, wv, bv, gamma, bn_w, bn_b)
    res = run_bass_kernel_spmd(
        nc, in_maps, core_ids=list(range(N_CORES)), trace=_trace
    )
    out = np.empty((B, CX, HX, WX), np.float32)
    for b in range(B):
        o2 = res.results[b]["out"].reshape(2, CX, SX // 2)
        out[b] = o2.transpose(1, 0, 2).reshape(CX, HX, WX)
    if _trace:
        _NC_CACHE["last_results"] = res
    return out



# revision 51
# speedup vs baseline: 1.0187x; 1.0187x over previous
"""Trainium2 Bass kernel for nn_Co_Pam_Module (PAM-style sparse attention +
nearest-upsample + BatchNorm residual).

Sharding: data-parallel over batch B=8 across 8 NeuronCores (one batch per
core); BN batch statistics are synchronized with a tiny AllReduce.

Math (validated vs reference, rel err ~1e-6 in numpy):
  q = wq@y + bq            [32, 2048]
  k = wk@y + bk            [32, 2048]
  E^T[t,s] = sum_d k[d,t] q[d,s]        (energy transposed; range ~±31 so
  P^T = exp(E^T)                         no max-subtraction is needed in f32)
  x_pool[c,j] = sum_u x[c,4j+u]
  vmm = (gamma*wv) @ x_pool             (gamma folded into weights)
  O~g[c,i] = sum_t vmm^T[t,c]*P^T[t,i]  via matmul with vpT=[vmm^T | ones];
  s[i]    = column 64 of the same accumulation (softmax denominator)
  G = O~g/s + 4*gamma*bv ; sync-BN stats via AllReduce of (sum G, sum G^2)
  out = x + scale_c*G_rep4 + bias_c
"""

import numpy as np

import concourse.bass as bass
import concourse.tile as tile
from concourse import library_config, mybir
from concourse.vector_clock import ScopedClock

F32 = mybir.dt.float32
F32R = mybir.dt.float32r
AF = mybir.ActivationFunctionType
ALU = mybir.AluOpType

# "rdma2" is the fast path in theory (3-round XOR gather-doubling over
# remote DMAs, ~4us vs the collective's 15us constant overhead) but this
# container's walrus cannot encode the SWDGE ISA structs (RemoteDMA*Descs,
# TriggerDma all fail codegen with "ISA wrong length"), so the collective
# is the only compilable cross-core sync.
SYNC_MODE = "collective"  # "rdma2" | "collective" | "none"

B, CX, HX, WX = 8, 64, 128, 64
CY, HY, WY = 256, 64, 32
SX, SY, D, RUP = HX * WX, HY * WY, 32, 4  # 8192, 2048, 32, 4
N_CORES = 8
BN_EPS = 1e-5
WPK_COLS = 968


# ---------------------------------------------------------------------------
# Workaround: walrus in this container rejects >cap sem waits on the Tile
# kernel-tail Drain.  Emit explicit per-sem wait_ge instructions instead.
def _patched_drain_and_barrier(self, tick_clock, wait_clock):
    nc = self.nc
    probe = nc.sync.nop(nofuse=True)
    wait_clock.add_sem_waits(probe.ins, ScopedClock({None: tick_clock.global_clock}))
    waits = list(probe.ins.sync_info.on_wait)
    probe.ins.sync_info.on_wait = []
    name2handle = {}
    for k, h in wait_clock.sems.allocated().items():
        name2handle[getattr(h, "name", str(k))] = h
    for w in waits:
        h = name2handle.get(w.ant_name)
        if h is None:
            raise RuntimeError(f"no sem handle for {w.ant_name}")
        nc.sync.wait_ge(h, w.wait_value)
    nc.sync.drain()
    nc.all_engine_barrier()
    popped = nc._tile_sem_poison_stack.pop()
    assert popped is self._sem_poison
    nc.clear_and_free_semaphores(list(self.sems.allocated().values()))
    nc.all_engine_barrier()


tile.TileContext._drain_and_barrier = _patched_drain_and_barrier


def _split_excess_waits(nc, cap=1):
    """Walrus in this container allows only `cap` sem waits per instruction.
    Hoist excess semaphore waits onto same-engine NoOps inserted just before
    the instruction (same engine + program order => semantics preserved)."""
    n_split = 0
    for f in nc.m.functions:
        for blk in f.blocks:
            insts = list(blk.instructions)
            new_insts = []
            changed = False
            for inst in insts:
                si = inst.sync_info
                waits = list(si.on_wait) if si is not None else []
                if len(waits) > cap:
                    sem_w = [w for w in waits if w.sync_type == "semaphore"]
                    other_w = [w for w in waits if w.sync_type != "semaphore"]
                    budget = max(0, cap - len(other_w))
                    keep, excess = sem_w[:budget], sem_w[budget:]
                    for i in range(0, len(excess), max(1, cap)):
                        chunk = excess[i : i + max(1, cap)]
                        nop = mybir.InstNoOp(
                            name=f"{inst.name}-ws{n_split}",
                            sync_info=mybir.SyncInfo(on_wait=chunk, on_update=[]),
                            bass_nofuse=True,
                            engine=inst.engine,
                        )
                        new_insts.append(nop)
                        n_split += 1
                    si.on_wait = other_w + keep
                    changed = True
                new_insts.append(inst)
            if changed:
                blk.instructions = new_insts
    return n_split
# ---------------------------------------------------------------------------


def _rep_ap(ap, rep):
    """Append a step-0 (repeat) innermost free dim to an AP."""
    return bass.AP(tensor=ap.tensor, offset=ap.offset, ap=list(ap.ap) + [[0, rep]])


def build_module(split_waits=True, sim_sync=False):
    nc = bass.Bass()

    xb = nc.dram_tensor("xb", [128, SX // 2], F32, kind="ExternalInput")
    yb = nc.dram_tensor("yb", [2, 128, SY], F32R, kind="ExternalInput")
    # packed small weights, one DMA: see _host_inputs for the column map
    wpk = nc.dram_tensor("wpk", [128, WPK_COLS], F32R, kind="ExternalInput")
    msc = nc.dram_tensor("msc", [128, 8], F32, kind="ExternalInput")
    out = nc.dram_tensor("out", [128, SX // 2], F32, kind="ExternalOutput")

    with tile.TileContext(nc, num_cores=N_CORES) as tc:
        with (
            tc.tile_pool(name="const", bufs=1) as cp,
            tc.tile_pool(name="big", bufs=1) as big,
            tc.tile_pool(name="ptile", bufs=10) as pp,
            tc.tile_pool(name="dram", bufs=1, space="DRAM") as dp,
        ):
            # ---------------- constants / weights (three DMA pieces) ---------
            wpk_sb = cp.tile([128, WPK_COLS], F32R)
            # column map (f32 cols): 0:256 wqT(kc0,kc1), 256:384 bq row,
            # 384:640 wkT, 768:832 wvT*gamma (stacked twice on partitions),
            # 840:968 repW. Pieces sized so the q0 matmul chain unblocks as
            # early as possible: A=[0:384] (wq+bq), B=[384:640] (wk),
            # C=[640:968] (wv+repW).
            bq_sb = wpk_sb[0:1, 256:384]
            bk_sb = None  # key bias dropped (softmax invariance)
            wv_sb = wpk_sb[:, 768:832]
            nc.sync.dma_start(wpk_sb[:, 0:384], wpk[:, 0:384])
            msc_sb = cp.tile([128, 8], F32)
            bv4g_sb = msc_sb[0:64, 0:1]
            bv4g_sb2 = msc_sb[:, 0:1]  # [128,1]
            c_s1_sb = msc_sb[0:64, 1:2]
            bv4g2_sb = msc_sb[0:64, 2:3]
            c_s2_sb = msc_sb[0:64, 3:4]
            bnw_sb = msc_sb[:, 4:5]  # [128,1] both halves
            bnb_sb = msc_sb[:, 5:6]  # [128,1] both halves

            # prewarm the PE clock (HAM ramp): memset FIRST so the dummy
            # matmul chain starts as early as possible — full PE speed needs
            # 3us of continuous execution before the q/k/E matmuls
            pewarm = cp.tile([128, 512], F32R)
            nc.vector.memset(pewarm[:].bitcast(F32), 0.0)

            ones_row = cp.tile([1, 512], F32R)
            nc.vector.memset(ones_row[:].bitcast(F32), 1.0)
            ones64 = cp.tile([1, 64], F32R)
            nc.vector.memset(ones64[:].bitcast(F32), 1.0)
            eps_sb = cp.tile([128, 1], F32)
            nc.vector.memset(eps_sb[:], BN_EPS)

            # prewarm exp table early (overlaps initial DMA)
            warm = cp.tile([1, 8], F32)
            nc.vector.memset(warm[:], 0.0)
            nc.scalar.activation(warm[:], warm[:], AF.Exp)

            # ---------------- big inputs ----------------
            y_sb = big.tile([128, 2, SY], F32R)
            # x in split layout: partition h*64+c holds x[c, 4096h:4096(h+1)]
            x2 = big.tile([128, SX // 2], F32)
            NXP = 4  # x pieces; pooling/vpT chunked to chase the DMA
            def y_quarter(ch):
                for kc in range(2):
                    nc.sync.dma_start(
                        y_sb[:, kc, ch * 512 : (ch + 1) * 512],
                        yb[kc][:, ch * 512 : (ch + 1) * 512],
                    )

            def x_piece(p):
                xsl = slice(p * 1024, (p + 1) * 1024)
                nc.sync.dma_start(x2[:, xsl], xb[:, xsl])

            # ordered so each consumer's operand lands just before its first
            # use: wpk piece A precedes this block (q0 chain), piece B (wk)
            # lands before k0, piece C (wv) before pool_sub(0); x pieces
            # chase the vpT chain (iter 2p), late y quarters only gate E at
            # iters 8/12
            y_quarter(0)
            y_quarter(1)
            nc.sync.dma_start(wpk_sb[:, 384:640], wpk[:, 384:640])
            nc.sync.dma_start(wpk_sb[:, 640:], wpk[:, 640:])
            x_piece(0)
            x_piece(1)
            y_quarter(2)
            x_piece(2)
            x_piece(3)
            y_quarter(3)
            nc.sync.dma_start(msc_sb[:], msc[:])

            # sync-BN allreduce buffers + pre-generated rdma descriptors.
            # XOR gather-doubling: round r sends buf[:, 0:2^(r+1)] to peer
            # me^(2^r), landing at buf[:, 2^(r+1):2^(r+2)]; after 3 rounds all
            # 8 cores' [128,2] stats sit in buf's 8 slots.  Descriptor
            # generation (~1us each on Pool SEQ) is hoisted here so only the
            # trigger/flight/ack path remains on the post-loop critical path.
            # Recv slots (cols 2:16) are remote-written only — no local writes,
            # no init (cross-core W-after-W races otherwise).
            if SYNC_MODE == "rdma2":
                arbuf = cp.tile([128, 16], F32)
                arbuf2 = cp.tile([128, 16], F32)
                ar_rsems = [nc.alloc_semaphore(f"ar_r{r}") for r in range(3)]
                ar_lsem = nc.alloc_semaphore("ar_l")
                ar_psem = nc.alloc_semaphore("ar_prep")
                with tc.tile_critical():
                    g = nc.gpsimd
                    g.load_library(library_config.remote_dma)
                    for r in range(3):
                        delta = 1 << r
                        slot = 4 if (delta & 4) else 0
                        rd = [None] * 8
                        rd[slot] = (0, delta)
                        g.remote_dma_broadcast(
                            arbuf[:, 2 * (1 << r) : 2 * (1 << (r + 1))],
                            arbuf[:, 0 : 2 * (1 << r)],
                            remote_sem=ar_rsems[r],
                            local_sem=ar_lsem,
                            rdests=rd,
                        ).then_inc(ar_psem, 1)

            q_sb = big.tile([128, SY], F32R)
            k_sb = big.tile([128, SY], F32R)

            # ---------------- main compute: single PSUM regime ----------------
            # psE: 3 rotating [128,1024] slots (6 banks) shared by warmup/qk/
            # vpT/rs-broadcast/E tiles; psO: [65,1024] accumulator (2 banks).
            # h=1's O accumulation reuses the psO slot and therefore waits for
            # half-0's recip/bcast/mult reads (~2.9us); the 8-deep P pool lets
            # the E/exp pipeline run ahead so that stall is absorbed.
            t1 = big.tile([128, SX // 4], F32)
            xp = big.tile([128, SX // 8], F32R)
            xv = x2[:].rearrange("p (n u) -> p n u", u=2)
            tv = t1[:].rearrange("p (n u) -> p n u", u=2)
            vpT = big.tile([128, 16, 65], F32R)
            nc.vector.memset(vpT[:, :, 64:65].bitcast(F32), 1.0)
            G0 = big.tile([64, SY], F32)
            s1_h = cp.tile([64, 2], F32)
            s2_h = cp.tile([64, 2], F32)
            junk2 = big.tile([64, 1024], F32, tag="junk2")
            junk3 = big.tile([64, 1024], F32, tag="junk3")
            with (
                tc.tile_pool(name="psE", bufs=3, space="PSUM") as psE,
                tc.tile_pool(name="psO", bufs=1, space="PSUM") as psO,
            ):
                # PE clock warmup — chain sized to bridge the gap until the
                # first q0 matmul's y data lands, so the ramp never resets and
                # the whole q/k/E chain runs at full clock
                wslot = psE.tile([128, 1024], F32, tag="E")
                for _ in range(6):
                    nc.tensor.matmul(
                        wslot[:, 0:512], pewarm[:, 0:128], pewarm[:],
                        start=True, stop=True,
                    )

                def emit_qk(w_off, b_t, dst, qt, with_bias=True):
                    # Key bias is dropped (with_bias=False for wk): softmax
                    # over t is invariant to the per-s column shift bk.q_s, so
                    # k = wk@y suffices — saves a 512-cycle PE matmul per
                    # quarter.
                    gslc = slice(qt * 512, (qt + 1) * 512)
                    ps = psE.tile([128, 1024], F32, tag="E")
                    for kc in range(2):
                        nc.tensor.matmul(
                            ps[:, 0:512],
                            wpk_sb[:, w_off + kc * 128 : w_off + kc * 128 + 128],
                            y_sb[:, kc, gslc],
                            start=(kc == 0),
                            stop=(kc == 1 and not with_bias),
                        )
                    if with_bias:
                        nc.tensor.matmul(
                            ps[:, 0:512], b_t[:], ones_row[:],
                            start=False, stop=True,
                        )
                    nc.vector.tensor_copy(dst[:, gslc], ps[:, 0:512])

                emit_qk(0, bq_sb, q_sb, 0)
                emit_qk(0, bq_sb, q_sb, 1)
                emit_qk(384, bk_sb, k_sb, 0, with_bias=False)

                def emit_pool_sub(jc):
                    # one 128-col xp window -> vpT chunks {2jc, 2jc+1}; spreads
                    # the piece work over two iterations to balance PE vs ACT
                    t1s = slice(jc * 256, (jc + 1) * 256)
                    nc.vector.tensor_add(t1[:, t1s], xv[:, t1s, 0], xv[:, t1s, 1])
                    xps = slice(jc * 128, (jc + 1) * 128)
                    nc.vector.tensor_add(xp[:, xps], tv[:, xps, 0], tv[:, xps, 1])
                    vps = psE.tile([128, 1024], F32, tag="E")
                    for hh in range(2):
                        base = slice(hh * 64, hh * 64 + 64)
                        # separate banks (cols 0 / 512): start=True bank clears
                        # cannot collide
                        nc.tensor.matmul(
                            vps[0:128, hh * 512 : hh * 512 + 64],
                            xp[base, xps], wv_sb[base, :],
                            start=True, stop=True,
                        )
                    vv = vps[:].rearrange("p (g c) -> p g c", c=512)
                    nc.vector.tensor_copy(vpT[:, 2 * jc : 2 * jc + 2, 0:64], vv[:, :, 0:64])

                def emit_half_tail(h, o_ps):
                    # walrus allows only ONE PSUM input per vector op, so rr
                    # bounces through SBUF.  h=0 runs hidden in-loop on DVE
                    # slack; h=1 is the critical post-loop path, pipelined in
                    # 2 column chunks across DVE (recip/mult/reduce), PE
                    # (broadcast) and ACT (rr copy + square accum).
                    isl_g = slice(h * 1024, (h + 1) * 1024)
                    rs_sb = big.tile([1, 1024], F32R, tag=f"rs{h}")
                    rr_ps = psE.tile([128, 1024], F32, tag="E")
                    rr_sb = big.tile([64, 1024], F32, tag=f"rsrep{h}")
                    if h == 0:
                        with nc.allow_low_precision(reason="fp32r softmax denom"):
                            nc.vector.reciprocal(rs_sb[:], o_ps[64:65, :])
                        for c2 in range(2):
                            nc.tensor.matmul(
                                rr_ps[0:64, c2 * 512 : (c2 + 1) * 512],
                                ones64[:],
                                rs_sb[:, c2 * 512 : (c2 + 1) * 512],
                                start=True, stop=True,
                            )
                        nc.vector.tensor_copy(rr_sb[:], rr_ps[0:64, :])
                        nc.vector.tensor_mul(G0[:, isl_g], o_ps[0:64, :], rr_sb[:])
                        nc.vector.tensor_reduce(
                            s1_h[:, h : h + 1], G0[:, isl_g],
                            mybir.AxisListType.X, ALU.add,
                        )
                    else:
                        with nc.allow_low_precision(reason="fp32r denom"):
                            nc.vector.reciprocal(rs_sb[:], o_ps[64:65, :])
                        for c2 in range(2):
                            nc.tensor.matmul(
                                rr_ps[0:64, c2 * 512 : (c2 + 1) * 512],
                                ones64[:],
                                rs_sb[:, c2 * 512 : (c2 + 1) * 512],
                                start=True, stop=True,
                            )
                        o_sb = big.tile([64, 1024], F32, tag="o1_sb")
                        nc.scalar.activation(o_sb[:], o_ps[0:64, :], AF.Copy)
                        nc.vector.tensor_mul(G0[:, isl_g], o_sb[:], rr_ps[0:64, :])
                        nc.vector.tensor_reduce(
                            s1_h[:, 1:2], G0[:, isl_g],
                            mybir.AxisListType.X, ALU.add,
                        )
                        nc.scalar.activation(
                            junk2[:], G0[:, isl_g], AF.Square,
                            accum_out=s2_h[:, 1:2],
                        )

                def emit_half_sq(h):
                    # half-0 squares on the otherwise-idle Pool engine,
                    # row-sum on DVE slack — fully hidden under the h=1
                    # loop (G0-lo is ready then); half-1's squares are fused
                    # into emit_half_tail(1)'s chunk pipeline.
                    assert h == 0
                    isl_g = slice(h * 1024, (h + 1) * 1024)
                    nc.gpsimd.tensor_mul(junk3[:], G0[:, isl_g], G0[:, isl_g])
                    nc.vector.tensor_reduce(
                        s2_h[:, 0:1], junk3[:], mybir.AxisListType.X, ALU.add
                    )

                o_ps_prev = None
                for h in range(2):
                    o_ps = psO.tile([65, 1024], F32, tag="O")
                    for ts in range(16):
                        tslc = slice(ts * 128, (ts + 1) * 128)
                        e_ps = psE.tile([128, 1024], F32, tag="E")
                        for c2 in range(2):
                            nc.tensor.matmul(
                                e_ps[:, c2 * 512 : (c2 + 1) * 512],
                                k_sb[0:32, tslc],
                                q_sb[0:32, h * 1024 + c2 * 512 : h * 1024 + c2 * 512 + 512],
                                start=True, stop=True,
                            )
                        p_sb = pp.tile([128, 1024], F32R, tag="P")
                        nc.scalar.activation(p_sb[:], e_ps[:], AF.Exp)
                        if h == 0:
                            if ts < 8:
                                emit_pool_sub(ts)
                            if ts == 1:
                                emit_qk(384, bk_sb, k_sb, 1, with_bias=False)
                            elif ts == 5:
                                emit_qk(384, bk_sb, k_sb, 2, with_bias=False)
                            elif ts == 9:
                                emit_qk(384, bk_sb, k_sb, 3, with_bias=False)
                            elif ts == 10:
                                emit_qk(0, bq_sb, q_sb, 2)
                            elif ts == 12:
                                emit_qk(0, bq_sb, q_sb, 3)

                        for c2 in range(2):
                            nc.tensor.matmul(
                                o_ps[:, c2 * 512 : (c2 + 1) * 512],
                                vpT[:, 2 * (ts % 8) + ts // 8, :],
                                p_sb[:, c2 * 512 : (c2 + 1) * 512],
                                start=(ts == 0),
                                stop=(ts == 15),
                                skip_group_check=True,
                            )
                        if h == 1 and ts == 1 and o_ps_prev is not None:
                            emit_half_tail(0, o_ps_prev)
                        if h == 1 and ts == 6:
                            emit_half_sq(0)  # Pool engine, hidden in-loop
                    o_ps_prev = o_ps
                emit_half_tail(1, o_ps_prev)

            # ---------------- BN stats + AllReduce ----------------
            # Raw per-core sums only: s1 = sum_i G~[c,i], s2 = sum_i G~^2.
            # BN is invariant to the constant shift bv4g (G_true = G~ + bv4g):
            # var = a2 - a1^2 and bias2 = bn_b - scale*a1 with a1,a2 the
            # globally-averaged raw sums — no bias correction terms needed.
            ar_sb = cp.tile([64, 2], F32R)
            nc.vector.tensor_add(ar_sb[:, 0:1], s1_h[:, 0:1], s1_h[:, 1:2])
            nc.vector.tensor_add(ar_sb[:, 1:2], s2_h[:, 0:1], s2_h[:, 1:2])

            # pre-move G0-hi to partitions 64:128 (hidden under the AllReduce)
            G2hi = big.tile([128, 1024], F32, tag="g2hi")
            nc.sync.dma_start(G2hi[64:128, :], G0[:, 1024:2048])

            if SYNC_MODE == "rdma2":
                # replicate [64,2] stats to all 128 partitions via PE
                # (repW = [I64|I64] lives in wpk cols 840:968)
                with tc.tile_pool(name="psT", bufs=1, space="PSUM") as psT:
                    rep_ps = psT.tile([128, 2], F32)
                    nc.tensor.matmul(
                        rep_ps[:], wpk_sb[0:64, 840:968], ar_sb[:],
                        start=True, stop=True,
                    )
                    nc.vector.tensor_copy(arbuf[:, 0:2], rep_ps[:])
                # warm the Sqrt table while the allreduce is in flight
                rs_warm = cp.tile([64, 1], F32)
                nc.scalar.activation(rs_warm[:], s1_h[:, 0:1], AF.Sqrt)
                with tc.tile_critical():
                    g = nc.gpsimd
                    scr = cp.tile([128, 2], F32, tag="rdma_scr")
                    g.wait_ge(ar_psem, 3)
                    g.tensor_copy(scr[:], arbuf[:, 0:2])  # order trigger after stats
                    for r in range(3):
                        g.trigger_dma(count=1)
                        if sim_sync:
                            # TimelineSim cannot deliver peer sem updates;
                            # stand in for flight+ack latency then self-satisfy
                            g.tensor_copy(scr[:], arbuf[:, 0:2])
                            g.sem_inc(ar_rsems[r], 2)
                        g.wait_ge(ar_rsems[r], 2)
                    g.tensor_copy(arbuf2[:], arbuf[:])  # local sync point
                sums_tile = cp.tile([128, 2], F32)
                nc.vector.tensor_reduce(
                    sums_tile[:],
                    arbuf2[:].rearrange("p (s c) -> p c s", c=2),
                    mybir.AxisListType.X,
                    ALU.add,
                )
                sums_sb = sums_tile[:]
            elif SYNC_MODE == "collective":
                ar_in = dp.tile([64, 2], F32)
                ar_out = dp.tile([N_CORES, 64, 2], F32)
                nc.sync.dma_start(ar_in[:], ar_sb[:].bitcast(F32))
                nc.gpsimd.collective_compute(
                    "AllGather",
                    ALU.bypass,
                    ins=[ar_in.opt()],
                    outs=[ar_out.opt()],
                    replica_groups=[list(range(N_CORES))],
                )
                # single gather onto partitions 0:64 as [64, 2, 8], rank-reduce
                gath_sb = cp.tile([64, 2, N_CORES], F32)
                nc.sync.dma_start(
                    gath_sb[:], ar_out[:].rearrange("r c j -> c j r")
                )
                sums_tile = cp.tile([64, 2], F32)
                nc.vector.tensor_reduce(
                    sums_tile[:], gath_sb[:], mybir.AxisListType.X, ALU.add
                )
                sums_sb = sums_tile[:]
            else:
                # debug fallback: per-core stats scaled by B (exact only if all
                # batches had identical stats)
                sums_tile = cp.tile([64, 2], F32)
                bounce = dp.tile([64, 2], F32)
                nc.sync.dma_start(bounce[:], ar_sb[:].bitcast(F32))
                nc.sync.dma_start(sums_tile[:], bounce[:])
                nc.vector.tensor_scalar_mul(sums_tile[:], sums_tile[:], float(B))
                sums_sb = sums_tile[:]

            # ---------------- scale/bias (64-wide) + PE replication ----------
            # From raw allreduced sums S1,S2 with c = 1/(B*SY):
            #   var = c*(S2 - c*S1^2);  scale = bn_w/sqrt(var+eps);
            #   bias2 = bn_b - scale*c*S1
            # (BN is invariant to the bv4g shift, so no bias corrections.)
            cnorm = float(RUP) / (B * SX)
            S1_ap = sums_sb[:, 0:1]
            S2_ap = sums_sb[:, 1:2]
            m2_sb = cp.tile([64, 1], F32)
            nc.vector.tensor_mul(m2_sb[:], S1_ap, S1_ap)
            w_sb = cp.tile([64, 1], F32)  # S2 - c*S1^2
            nc.vector.tensor_scalar(
                w_sb[:], m2_sb[:], -cnorm, S2_ap, ALU.mult, ALU.add
            )
            std_sb = cp.tile([64, 1], F32)
            nc.scalar.activation(
                std_sb[:], w_sb[:], AF.Sqrt, bias=eps_sb[0:64, :], scale=cnorm
            )
            rstd_sb = cp.tile([64, 1], F32)
            nc.vector.reciprocal(rstd_sb[:], std_sb[:])
            sb2 = cp.tile([64, 2], F32R)  # (scale, bias2) packed for PE rep
            nc.vector.tensor_mul(sb2[:, 0:1], rstd_sb[:], bnw_sb[0:64, :])
            u_sb = cp.tile([64, 1], F32)
            nc.vector.tensor_mul(u_sb[:], S1_ap, sb2[:, 0:1])
            nc.vector.tensor_scalar(
                sb2[:, 1:2], u_sb[:], -cnorm, bnb_sb[0:64, :], ALU.mult, ALU.add
            )
            # replicate (scale, bias2) to all 128 partitions via repW matmul
            with tc.tile_pool(name="psT", bufs=1, space="PSUM") as psT:
                rep_ps = psT.tile([128, 2], F32)
                nc.tensor.matmul(
                    rep_ps[:], wpk_sb[0:64, 840:968], sb2[:],
                    start=True, stop=True,
                )
                sb128 = cp.tile([128, 2], F32)
                nc.vector.tensor_copy(sb128[:], rep_ps[:])

            # R2 split layout [128, 1024]: R2[h*64+c, i'] = scale*G[...]+bias2.
            # Lower half on ACT (out = func(in*scale+bias)), upper half on DVE
            # tensor_scalar — the two engines run in parallel; 2 column chunks
            # each so the combine can start early.
            # G0-hi was pre-moved to partitions 64:128 under the AllGather.
            R2 = big.tile([128, 1024], F32)
            r2_bounds = [0, 128, 384, 704, 1024]  # small first chunk so the
            for rc in range(4):                   # out-DMA stream starts early
                rsl = slice(r2_bounds[rc], r2_bounds[rc + 1])
                nc.scalar.activation(
                    R2[0:64, rsl], G0[:, rsl], AF.Identity,
                    scale=sb128[0:64, 0:1],
                    bias=sb128[0:64, 1:2],
                )
                nc.vector.tensor_scalar(
                    R2[64:128, rsl], G2hi[64:128, rsl], sb128[64:128, 0:1],
                    sb128[64:128, 1:2], ALU.mult, ALU.add,
                )

            # out2[p, f] = x2[p, f] + R2[p, f>>2]; 8 compute chunks split
            # 5 DVE / 3 Pool (Pool is ~2x slower per element), DMAs grouped
            # two chunks each to halve HWDGE serialization.
            out2 = big.tile([128, SX // 2], F32)
            o_view = out2[:].rearrange("p (n u) -> p n u", u=4)
            x_view = x2[:].rearrange("p (n u) -> p n u", u=4)
            NFC = 8
            csz = (SX // 2) // NFC  # 512 output cols -> 128 R cols per chunk
            pool_chunks = {1, 4, 7}
            for qc in range(NFC):
                nsl = slice(qc * (csz // 4), (qc + 1) * (csz // 4))
                eng = nc.gpsimd if qc in pool_chunks else nc.vector
                eng.tensor_add(
                    o_view[:, nsl, :],
                    x_view[:, nsl, :],
                    _rep_ap(R2[:, nsl], 4),
                )
                nc.sync.dma_start(
                    out[:, qc * csz : (qc + 1) * csz],
                    out2[:, qc * csz : (qc + 1) * csz],
                )

    if split_waits:
        _split_excess_waits(nc)
    return nc


def _host_inputs(x, y, wq, bq, wk, bk, wv, bv, gamma, bn_w, bn_b):
    g = float(np.asarray(gamma).reshape(-1)[0])
    wqT_rep = np.tile(np.ascontiguousarray(wq.T), (1, 4))  # [256, 128]
    wkT_rep = np.tile(np.ascontiguousarray(wk.T), (1, 4))
    bv4g = (4.0 * g * bv)
    wpk = np.zeros((128, WPK_COLS), np.float32)
    wpk[:, 0:128] = wqT_rep[0:128]
    wpk[:, 128:256] = wqT_rep[128:256]
    wpk[0, 256:384] = np.tile(bq, 4)
    wpk[:, 384:512] = wkT_rep[0:128]
    wpk[:, 512:640] = wkT_rep[128:256]
    # bk is dropped on-device: softmax over t is invariant to the key bias
    wpk[0:64, 768:832] = (g * wv).T
    wpk[64:128, 768:832] = (g * wv).T
    # repW: [64,128] with W[c,p]=1 iff p%64==c — PE partition replication
    wpk[0:64, 840:968] = np.tile(np.eye(64, dtype=np.float32), (1, 2))
    msc = np.zeros((128, 8), np.float32)
    for hh in range(2):
        msc[hh * 64 : hh * 64 + 64, 0] = bv4g
        msc[hh * 64 : hh * 64 + 64, 1] = SY * bv4g
        msc[hh * 64 : hh * 64 + 64, 2] = 2.0 * bv4g
        msc[hh * 64 : hh * 64 + 64, 3] = SY * bv4g * bv4g
        msc[hh * 64 : hh * 64 + 64, 4] = bn_w
        msc[hh * 64 : hh * 64 + 64, 5] = bn_b
    common = {"wpk": wpk, "msc": msc}
    in_maps = []
    for b in range(B):
        m = dict(common)
        # split layout: [2, 64, 4096] where [h, c, f] = x[b, c, 4096h + f]
        xf = np.asarray(x[b], np.float32).reshape(64, 2, SX // 2).transpose(1, 0, 2)
        m["xb"] = np.ascontiguousarray(xf.reshape(128, SX // 2))
        m["yb"] = np.ascontiguousarray(
            np.asarray(y[b], np.float32).reshape(2, 128, SY)
        )
        in_maps.append(m)
    return in_maps


_NC_CACHE = {}


def kernel(x, y, wq, bq, wk, bk, wv, bv, gamma, bn_w, bn_b, _trace=False):
    from concourse.bass_utils import run_bass_kernel_spmd

    if "nc" not in _NC_CACHE:
        _NC_CACHE["nc"] = build_module()
    nc = _NC_CACHE["nc"]
    in_maps = _host_inputs(x, y, wq, bq, wk, bk, wv, bv, gamma, bn_w, bn_b)
    res = run_bass_kernel_spmd(
        nc, in_maps, core_ids=list(range(N_CORES)), trace=_trace
    )
    out = np.empty((B, CX, HX, WX), np.float32)
    for b in range(B):
        o2 = res.results[b]["out"].reshape(2, CX, SX // 2)
        out[b] = o2.transpose(1, 0, 2).reshape(CX, HX, WX)
    if _trace:
        _NC_CACHE["last_results"] = res
    return out



# revision 65
# speedup vs baseline: 1.0497x; 1.0304x over previous
"""Trainium2 Bass kernel for nn_Co_Pam_Module (PAM-style sparse attention +
nearest-upsample + BatchNorm residual).

Sharding: data-parallel over batch B=8 across 8 NeuronCores (one batch per
core); BN batch statistics are synchronized with a tiny AllReduce.

Math (validated vs reference, rel err ~1e-6 in numpy):
  q = wq@y + bq            [32, 2048]
  k = wk@y + bk            [32, 2048]
  E^T[t,s] = sum_d k[d,t] q[d,s]        (energy transposed; range ~±31 so
  P^T = exp(E^T)                         no max-subtraction is needed in f32)
  x_pool[c,j] = sum_u x[c,4j+u]
  vmm = (gamma*wv) @ x_pool             (gamma folded into weights)
  O~g[c,i] = sum_t vmm^T[t,c]*P^T[t,i]  via matmul with vpT=[vmm^T | ones];
  s[i]    = column 64 of the same accumulation (softmax denominator)
  G = O~g/s + 4*gamma*bv ; sync-BN stats via AllReduce of (sum G, sum G^2)
  out = x + scale_c*G_rep4 + bias_c
"""

import numpy as np

import concourse.bass as bass
import concourse.tile as tile
from concourse import library_config, mybir
from concourse.vector_clock import ScopedClock

F32 = mybir.dt.float32
F32R = mybir.dt.float32r
BF16 = mybir.dt.bfloat16
AF = mybir.ActivationFunctionType
ALU = mybir.AluOpType

# "rdma2" is the fast path in theory (3-round XOR gather-doubling over
# remote DMAs, ~4us vs the collective's 15us constant overhead) but this
# container's walrus cannot encode the SWDGE ISA structs (RemoteDMA*Descs,
# TriggerDma all fail codegen with "ISA wrong length"), so the collective
# is the only compilable cross-core sync.
SYNC_MODE = "collective"  # "rdma2" | "collective" | "none"

B, CX, HX, WX = 8, 64, 128, 64
CY, HY, WY = 256, 64, 32
SX, SY, D, RUP = HX * WX, HY * WY, 32, 4  # 8192, 2048, 32, 4
N_CORES = 8
BN_EPS = 1e-5
WPK_COLS = 968


# ---------------------------------------------------------------------------
# Workaround: walrus in this container rejects >cap sem waits on the Tile
# kernel-tail Drain.  Emit explicit per-sem wait_ge instructions instead.
def _patched_drain_and_barrier(self, tick_clock, wait_clock):
    nc = self.nc
    probe = nc.sync.nop(nofuse=True)
    wait_clock.add_sem_waits(probe.ins, ScopedClock({None: tick_clock.global_clock}))
    waits = list(probe.ins.sync_info.on_wait)
    probe.ins.sync_info.on_wait = []
    name2handle = {}
    for k, h in wait_clock.sems.allocated().items():
        name2handle[getattr(h, "name", str(k))] = h
    for w in waits:
        h = name2handle.get(w.ant_name)
        if h is None:
            raise RuntimeError(f"no sem handle for {w.ant_name}")
        nc.sync.wait_ge(h, w.wait_value)
    nc.sync.drain()
    nc.all_engine_barrier()
    popped = nc._tile_sem_poison_stack.pop()
    assert popped is self._sem_poison
    nc.clear_and_free_semaphores(list(self.sems.allocated().values()))
    nc.all_engine_barrier()


tile.TileContext._drain_and_barrier = _patched_drain_and_barrier


def _split_excess_waits(nc, cap=1):
    """Walrus in this container allows only `cap` sem waits per instruction.
    Hoist excess semaphore waits onto same-engine NoOps inserted just before
    the instruction (same engine + program order => semantics preserved)."""
    n_split = 0
    for f in nc.m.functions:
        for blk in f.blocks:
            insts = list(blk.instructions)
            new_insts = []
            changed = False
            for inst in insts:
                si = inst.sync_info
                waits = list(si.on_wait) if si is not None else []
                if len(waits) > cap:
                    sem_w = [w for w in waits if w.sync_type == "semaphore"]
                    other_w = [w for w in waits if w.sync_type != "semaphore"]
                    budget = max(0, cap - len(other_w))
                    keep, excess = sem_w[:budget], sem_w[budget:]
                    for i in range(0, len(excess), max(1, cap)):
                        chunk = excess[i : i + max(1, cap)]
                        nop = mybir.InstNoOp(
                            name=f"{inst.name}-ws{n_split}",
                            sync_info=mybir.SyncInfo(on_wait=chunk, on_update=[]),
                            bass_nofuse=True,
                            engine=inst.engine,
                        )
                        new_insts.append(nop)
                        n_split += 1
                    si.on_wait = other_w + keep
                    changed = True
                new_insts.append(inst)
            if changed:
                blk.instructions = new_insts
    return n_split
# ---------------------------------------------------------------------------


def _rep_ap(ap, rep):
    """Append a step-0 (repeat) innermost free dim to an AP."""
    return bass.AP(tensor=ap.tensor, offset=ap.offset, ap=list(ap.ap) + [[0, rep]])


def build_module(split_waits=True, sim_sync=False):
    nc = bass.Bass()

    # bf16 x/y inputs halve the 4.25MB input stream — every DMA arrival
    # (and thus exp0 and the k-quarter emissions) moves ~2x earlier.
    xb = nc.dram_tensor("xb", [128, SX // 2], BF16, kind="ExternalInput")
    yb = nc.dram_tensor("yb", [2, 128, SY], BF16, kind="ExternalInput")
    # packed weights: bf16 piece (wq/bq/wk/wv — matmul partners of bf16
    # data) + f32r piece (repW, partner of the f32r stats matmul)
    wpk = nc.dram_tensor("wpk", [128, 704], BF16, kind="ExternalInput")
    wpkr = nc.dram_tensor("wpkr", [64, 128], F32R, kind="ExternalInput")
    msc = nc.dram_tensor("msc", [128, 8], F32, kind="ExternalInput")
    # bf16 output halves the 2MB store stream (~2.9us saved on the tail);
    # host upcasts to f32.  out = x + BN(...) in bf16 costs ~2e-3 rel err
    # against a 2e-2 budget.
    out = nc.dram_tensor("out", [128, SX // 2], BF16, kind="ExternalOutput")

    with tile.TileContext(nc, num_cores=N_CORES) as tc:
        with (
            tc.tile_pool(name="const", bufs=1) as cp,
            tc.tile_pool(name="big", bufs=1) as big,
            tc.tile_pool(name="ptile", bufs=12) as pp,
            tc.tile_pool(name="dram", bufs=1, space="DRAM") as dp,
        ):
            # ---------------- constants / weights (three DMA pieces) ---------
            wpk_sb = cp.tile([128, 704], BF16)
            wpkr_sb = cp.tile([64, 128], F32R)
            # bf16 column map: 0:256 wqT(kc0,kc1), 256:384 bq row, 384:640
            # wkT, 640:704 wvT*gamma (stacked twice on partitions).  Pieces
            # sized so the q0 matmul chain unblocks as early as possible:
            # A=[0:384] (wq+bq), B=[384:640] (wk), C=[640:704]+wpkr.
            bq_sb = wpk_sb[0:1, 256:384]
            bk_sb = None  # key bias dropped (softmax invariance)
            wv_sb = wpk_sb[:, 640:704]
            repW_sb = wpkr_sb[:, :]
            nc.sync.dma_start(wpk_sb[:, 0:384], wpk[:, 0:384])
            msc_sb = cp.tile([128, 8], F32)
            bv4g_sb = msc_sb[0:64, 0:1]
            bv4g_sb2 = msc_sb[:, 0:1]  # [128,1]
            c_s1_sb = msc_sb[0:64, 1:2]
            bv4g2_sb = msc_sb[0:64, 2:3]
            c_s2_sb = msc_sb[0:64, 3:4]
            bnw_sb = msc_sb[:, 4:5]  # [128,1] both halves
            bnb_sb = msc_sb[:, 5:6]  # [128,1] both halves

            # prewarm the PE clock (HAM ramp): memset FIRST so the dummy
            # matmul chain starts as early as possible — full PE speed needs
            # 3us of continuous execution before the q/k/E matmuls
            pewarm = cp.tile([128, 512], F32R)
            nc.vector.memset(pewarm[:].bitcast(F32), 0.0)

            ones_row = cp.tile([1, 512], BF16)
            nc.vector.memset(ones_row[:], 1.0)
            ones64 = cp.tile([1, 64], F32R)
            nc.vector.memset(ones64[:].bitcast(F32), 1.0)
            eps_sb = cp.tile([128, 1], F32)
            nc.vector.memset(eps_sb[:], BN_EPS)

            # prewarm exp table early (overlaps initial DMA)
            warm = cp.tile([1, 8], F32)
            nc.vector.memset(warm[:], 0.0)
            nc.scalar.activation(warm[:], warm[:], AF.Exp)

            # ---------------- big inputs ----------------
            y_sb = big.tile([128, 2, SY], BF16)
            # x in split layout: partition h*64+c holds x[c, 4096h:4096(h+1)]
            x2 = big.tile([128, SX // 2], BF16)
            NXP = 4  # x pieces; pooling/vpT chunked to chase the DMA
            def y_quarter(ch):
                for kc in range(2):
                    nc.sync.dma_start(
                        y_sb[:, kc, ch * 512 : (ch + 1) * 512],
                        yb[kc][:, ch * 512 : (ch + 1) * 512],
                    )

            def x_piece(p):
                xsl = slice(p * 1024, (p + 1) * 1024)
                nc.sync.dma_start(x2[:, xsl], xb[:, xsl])

            # ordered so each consumer's operand lands just before its first
            # use: wpk piece A precedes this block (q0 chain), piece B (wk)
            # lands before k0, piece C (wv) before pool_sub(0); x pieces
            # chase the vpT chain (iter 2p), late y quarters only gate E at
            # iters 8/12
            y_quarter(0)
            nc.sync.dma_start(wpk_sb[:, 384:640], wpk[:, 384:640])
            y_quarter(1)
            nc.sync.dma_start(wpk_sb[:, 640:], wpk[:, 640:])
            nc.sync.dma_start(wpkr_sb[:], wpkr[:])
            x_piece(0)
            x_piece(1)
            y_quarter(2)
            x_piece(2)
            x_piece(3)
            y_quarter(3)
            nc.sync.dma_start(msc_sb[:], msc[:])

            # sync-BN allreduce buffers + pre-generated rdma descriptors.
            # XOR gather-doubling: round r sends buf[:, 0:2^(r+1)] to peer
            # me^(2^r), landing at buf[:, 2^(r+1):2^(r+2)]; after 3 rounds all
            # 8 cores' [128,2] stats sit in buf's 8 slots.  Descriptor
            # generation (~1us each on Pool SEQ) is hoisted here so only the
            # trigger/flight/ack path remains on the post-loop critical path.
            # Recv slots (cols 2:16) are remote-written only — no local writes,
            # no init (cross-core W-after-W races otherwise).
            if SYNC_MODE == "rdma2":
                arbuf = cp.tile([128, 16], F32)
                arbuf2 = cp.tile([128, 16], F32)
                ar_rsems = [nc.alloc_semaphore(f"ar_r{r}") for r in range(3)]
                ar_lsem = nc.alloc_semaphore("ar_l")
                ar_psem = nc.alloc_semaphore("ar_prep")
                with tc.tile_critical():
                    g = nc.gpsimd
                    g.load_library(library_config.remote_dma)
                    for r in range(3):
                        delta = 1 << r
                        slot = 4 if (delta & 4) else 0
                        rd = [None] * 8
                        rd[slot] = (0, delta)
                        g.remote_dma_broadcast(
                            arbuf[:, 2 * (1 << r) : 2 * (1 << (r + 1))],
                            arbuf[:, 0 : 2 * (1 << r)],
                            remote_sem=ar_rsems[r],
                            local_sem=ar_lsem,
                            rdests=rd,
                        ).then_inc(ar_psem, 1)

            q_sb = big.tile([128, SY], F32R)
            k_sb = big.tile([128, SY], F32R)

            # ---------------- main compute: single PSUM regime ----------------
            # psE: 3 rotating [128,1024] slots (6 banks) shared by warmup/qk/
            # vpT/rs-broadcast/E tiles; psO: [65,1024] accumulator (2 banks).
            # h=1's O accumulation reuses the psO slot and therefore waits for
            # half-0's recip/bcast/mult reads (~2.9us); the 8-deep P pool lets
            # the E/exp pipeline run ahead so that stall is absorbed.
            t1 = big.tile([128, SX // 4], BF16)
            xp = big.tile([128, SX // 8], BF16)
            xv = x2[:].rearrange("p (n u) -> p n u", u=2)
            tv = t1[:].rearrange("p (n u) -> p n u", u=2)
            vpT = big.tile([128, 16, 65], F32R)
            nc.vector.memset(vpT[:, :, 64:65].bitcast(F32), 1.0)
            G0 = big.tile([64, SY], F32)
            s1_h = cp.tile([64, 2], F32)
            s2_h = cp.tile([64, 2], F32)
            junk2 = big.tile([64, 1024], F32, tag="junk2")
            junk3 = big.tile([64, 1024], F32, tag="junk3")
            with (
                tc.tile_pool(name="psE", bufs=3, space="PSUM") as psE,
                tc.tile_pool(name="psO", bufs=1, space="PSUM") as psO,
            ):
                # PE clock warmup — chain sized to bridge the gap until the
                # first q0 matmul's y data lands, so the ramp never resets and
                # the whole q/k/E chain runs at full clock
                wslot = psE.tile([128, 1024], F32, tag="E")
                for _ in range(6):
                    nc.tensor.matmul(
                        wslot[:, 0:512], pewarm[:, 0:128], pewarm[:],
                        start=True, stop=True,
                    )

                def emit_qk(w_off, b_t, dst, qt, with_bias=True):
                    # Key bias is dropped (with_bias=False for wk): softmax
                    # over t is invariant to the per-s column shift bk.q_s, so
                    # k = wk@y suffices — saves a 512-cycle PE matmul per
                    # quarter.
                    gslc = slice(qt * 512, (qt + 1) * 512)
                    ps = psE.tile([128, 1024], F32, tag="E")
                    for kc in range(2):
                        nc.tensor.matmul(
                            ps[:, 0:512],
                            wpk_sb[:, w_off + kc * 128 : w_off + kc * 128 + 128],
                            y_sb[:, kc, gslc],
                            start=(kc == 0),
                            stop=(kc == 1 and not with_bias),
                        )
                    if with_bias:
                        nc.tensor.matmul(
                            ps[:, 0:512], b_t[:], ones_row[:],
                            start=False, stop=True,
                        )
                    nc.vector.tensor_copy(dst[:, gslc], ps[:, 0:512])

                emit_qk(0, bq_sb, q_sb, 0)
                emit_qk(384, bk_sb, k_sb, 0, with_bias=False)
                emit_qk(0, bq_sb, q_sb, 1)

                def emit_pool_sub(jc):
                    # one 128-col xp window -> vpT chunks {2jc, 2jc+1}; spreads
                    # the piece work over two iterations to balance PE vs ACT
                    t1s = slice(jc * 256, (jc + 1) * 256)
                    nc.vector.tensor_add(t1[:, t1s], xv[:, t1s, 0], xv[:, t1s, 1])
                    xps = slice(jc * 128, (jc + 1) * 128)
                    nc.vector.tensor_add(xp[:, xps], tv[:, xps, 0], tv[:, xps, 1])
                    vps = psE.tile([128, 1024], F32, tag="E")
                    for hh in range(2):
                        base = slice(hh * 64, hh * 64 + 64)
                        # separate banks (cols 0 / 512): start=True bank clears
                        # cannot collide
                        nc.tensor.matmul(
                            vps[0:128, hh * 512 : hh * 512 + 64],
                            xp[base, xps], wv_sb[base, :],
                            start=True, stop=True,
                        )
                    vv = vps[:].rearrange("p (g c) -> p g c", c=512)
                    nc.vector.tensor_copy(vpT[:, 2 * jc : 2 * jc + 2, 0:64], vv[:, :, 0:64])

                def emit_half_tail(h, o_ps):
                    # walrus allows only ONE PSUM input per vector op, so rr
                    # bounces through SBUF.  h=0 runs hidden in-loop on DVE
                    # slack; h=1 is the critical post-loop path, pipelined in
                    # 2 column chunks across DVE (recip/mult/reduce), PE
                    # (broadcast) and ACT (rr copy + square accum).
                    isl_g = slice(h * 1024, (h + 1) * 1024)
                    rs_sb = big.tile([1, 1024], F32R, tag=f"rs{h}")
                    rr_ps = psE.tile([128, 1024], F32, tag="E")
                    rr_sb = big.tile([64, 1024], F32, tag=f"rsrep{h}")
                    if h == 0:
                        with nc.allow_low_precision(reason="fp32r softmax denom"):
                            nc.vector.reciprocal(rs_sb[:], o_ps[64:65, :])
                        # copy O (not rr) out of PSUM: releases the psO
                        # accumulator ~2us earlier so h=1's O matmuls unblock
                        o0_sb = big.tile([64, 1024], F32, tag="o0_sb")
                        nc.vector.tensor_copy(o0_sb[:], o_ps[0:64, :])
                        for c2 in range(2):
                            nc.tensor.matmul(
                                rr_ps[0:64, c2 * 512 : (c2 + 1) * 512],
                                ones64[:],
                                rs_sb[:, c2 * 512 : (c2 + 1) * 512],
                                start=True, stop=True,
                            )
                        nc.vector.tensor_mul(G0[:, isl_g], o0_sb[:], rr_ps[0:64, :])
                        nc.vector.tensor_reduce(
                            s1_h[:, h : h + 1], G0[:, isl_g],
                            mybir.AxisListType.X, ALU.add,
                        )
                    else:
                        with nc.allow_low_precision(reason="fp32r denom"):
                            nc.vector.reciprocal(rs_sb[:], o_ps[64:65, :])
                        for c2 in range(2):
                            nc.tensor.matmul(
                                rr_ps[0:64, c2 * 512 : (c2 + 1) * 512],
                                ones64[:],
                                rs_sb[:, c2 * 512 : (c2 + 1) * 512],
                                start=True, stop=True,
                            )
                        o_sb = big.tile([64, 1024], F32, tag="o1_sb")
                        nc.scalar.activation(o_sb[:], o_ps[0:64, :], AF.Copy)
                        nc.vector.tensor_mul(G0[:, isl_g], o_sb[:], rr_ps[0:64, :])
                        nc.vector.tensor_reduce(
                            s1_h[:, 1:2], G0[:, isl_g],
                            mybir.AxisListType.X, ALU.add,
                        )
                        nc.scalar.activation(
                            junk2[:], G0[:, isl_g], AF.Square,
                            accum_out=s2_h[:, 1:2],
                        )

                def emit_half_sq(h):
                    # half-0 squares on the otherwise-idle Pool engine,
                    # row-sum on DVE slack — fully hidden under the h=1
                    # loop (G0-lo is ready then); half-1's squares are fused
                    # into emit_half_tail(1)'s chunk pipeline.
                    assert h == 0
                    isl_g = slice(h * 1024, (h + 1) * 1024)
                    nc.gpsimd.tensor_mul(junk3[:], G0[:, isl_g], G0[:, isl_g])
                    nc.vector.tensor_reduce(
                        s2_h[:, 0:1], junk3[:], mybir.AxisListType.X, ALU.add
                    )

                o_ps_prev = None
                for h in range(2):
                    o_ps = psO.tile([65, 1024], F32, tag="O")
                    for ts in range(16):
                        tslc = slice(ts * 128, (ts + 1) * 128)
                        e_ps = psE.tile([128, 1024], F32, tag="E")
                        for c2 in range(2):
                            nc.tensor.matmul(
                                e_ps[:, c2 * 512 : (c2 + 1) * 512],
                                k_sb[0:32, tslc],
                                q_sb[0:32, h * 1024 + c2 * 512 : h * 1024 + c2 * 512 + 512],
                                start=True, stop=True,
                            )
                        p_sb = pp.tile([128, 1024], F32R, tag="P")
                        nc.scalar.activation(p_sb[:], e_ps[:], AF.Exp)
                        if h == 0:
                            if ts < 8:
                                emit_pool_sub(ts)
                            if ts == 1:
                                emit_qk(384, bk_sb, k_sb, 1, with_bias=False)
                            elif ts == 5:
                                emit_qk(384, bk_sb, k_sb, 2, with_bias=False)
                            elif ts == 9:
                                emit_qk(384, bk_sb, k_sb, 3, with_bias=False)
                            elif ts == 10:
                                emit_qk(0, bq_sb, q_sb, 2)
                            elif ts == 12:
                                emit_qk(0, bq_sb, q_sb, 3)

                        for c2 in range(2):
                            nc.tensor.matmul(
                                o_ps[:, c2 * 512 : (c2 + 1) * 512],
                                vpT[:, 2 * (ts % 8) + ts // 8, :],
                                p_sb[:, c2 * 512 : (c2 + 1) * 512],
                                start=(ts == 0),
                                stop=(ts == 15),
                                skip_group_check=True,
                            )
                        if h == 1 and ts == 1 and o_ps_prev is not None:
                            emit_half_tail(0, o_ps_prev)
                        if h == 1 and ts == 6:
                            emit_half_sq(0)  # Pool engine, hidden in-loop
                    o_ps_prev = o_ps
                emit_half_tail(1, o_ps_prev)

            # ---------------- BN stats + AllReduce ----------------
            # Raw per-core sums only: s1 = sum_i G~[c,i], s2 = sum_i G~^2.
            # BN is invariant to the constant shift bv4g (G_true = G~ + bv4g):
            # var = a2 - a1^2 and bias2 = bn_b - scale*a1 with a1,a2 the
            # globally-averaged raw sums — no bias correction terms needed.
            ar_sb = cp.tile([64, 2], F32R)
            nc.vector.tensor_add(ar_sb[:, 0:1], s1_h[:, 0:1], s1_h[:, 1:2])
            nc.vector.tensor_add(ar_sb[:, 1:2], s2_h[:, 0:1], s2_h[:, 1:2])

            # pre-move G0-hi to partitions 64:128 (hidden under the AllReduce)
            G2hi = big.tile([128, 1024], F32, tag="g2hi")
            nc.sync.dma_start(G2hi[64:128, :], G0[:, 1024:2048])

            if SYNC_MODE == "rdma2":
                # replicate [64,2] stats to all 128 partitions via PE
                # (repW = [I64|I64] lives in wpk cols 840:968)
                with tc.tile_pool(name="psT", bufs=1, space="PSUM") as psT:
                    rep_ps = psT.tile([128, 2], F32)
                    nc.tensor.matmul(
                        rep_ps[:], repW_sb[:], ar_sb[:],
                        start=True, stop=True,
                    )
                    nc.vector.tensor_copy(arbuf[:, 0:2], rep_ps[:])
                # warm the Sqrt table while the allreduce is in flight
                rs_warm = cp.tile([64, 1], F32)
                nc.scalar.activation(rs_warm[:], s1_h[:, 0:1], AF.Sqrt)
                with tc.tile_critical():
                    g = nc.gpsimd
                    scr = cp.tile([128, 2], F32, tag="rdma_scr")
                    g.wait_ge(ar_psem, 3)
                    g.tensor_copy(scr[:], arbuf[:, 0:2])  # order trigger after stats
                    for r in range(3):
                        g.trigger_dma(count=1)
                        if sim_sync:
                            # TimelineSim cannot deliver peer sem updates;
                            # stand in for flight+ack latency then self-satisfy
                            g.tensor_copy(scr[:], arbuf[:, 0:2])
                            g.sem_inc(ar_rsems[r], 2)
                        g.wait_ge(ar_rsems[r], 2)
                    g.tensor_copy(arbuf2[:], arbuf[:])  # local sync point
                sums_tile = cp.tile([128, 2], F32)
                nc.vector.tensor_reduce(
                    sums_tile[:],
                    arbuf2[:].rearrange("p (s c) -> p c s", c=2),
                    mybir.AxisListType.X,
                    ALU.add,
                )
                sums_sb = sums_tile[:]
            elif SYNC_MODE == "collective":
                ar_in = dp.tile([64, 2], F32)
                ar_out = dp.tile([N_CORES, 64, 2], F32)
                nc.sync.dma_start(ar_in[:], ar_sb[:].bitcast(F32))
                nc.gpsimd.collective_compute(
                    "AllGather",
                    ALU.bypass,
                    ins=[ar_in.opt()],
                    outs=[ar_out.opt()],
                    replica_groups=[list(range(N_CORES))],
                )
                # single gather onto partitions 0:64 as [64, 2, 8], rank-reduce
                gath_sb = cp.tile([64, 2, N_CORES], F32)
                nc.sync.dma_start(
                    gath_sb[:], ar_out[:].rearrange("r c j -> c j r")
                )
                sums_tile = cp.tile([64, 2], F32)
                nc.vector.tensor_reduce(
                    sums_tile[:], gath_sb[:], mybir.AxisListType.X, ALU.add
                )
                sums_sb = sums_tile[:]
            else:
                # debug fallback: per-core stats scaled by B (exact only if all
                # batches had identical stats)
                sums_tile = cp.tile([64, 2], F32)
                bounce = dp.tile([64, 2], F32)
                nc.sync.dma_start(bounce[:], ar_sb[:].bitcast(F32))
                nc.sync.dma_start(sums_tile[:], bounce[:])
                nc.vector.tensor_scalar_mul(sums_tile[:], sums_tile[:], float(B))
                sums_sb = sums_tile[:]

            # ---------------- scale/bias (64-wide) + PE replication ----------
            # From raw allreduced sums S1,S2 with c = 1/(B*SY):
            #   var = c*(S2 - c*S1^2);  scale = bn_w/sqrt(var+eps);
            #   bias2 = bn_b - scale*c*S1
            # (BN is invariant to the bv4g shift, so no bias corrections.)
            cnorm = float(RUP) / (B * SX)
            S1_ap = sums_sb[:, 0:1]
            S2_ap = sums_sb[:, 1:2]
            m2_sb = cp.tile([64, 1], F32)
            nc.vector.tensor_mul(m2_sb[:], S1_ap, S1_ap)
            w_sb = cp.tile([64, 1], F32)  # S2 - c*S1^2
            nc.vector.tensor_scalar(
                w_sb[:], m2_sb[:], -cnorm, S2_ap, ALU.mult, ALU.add
            )
            std_sb = cp.tile([64, 1], F32)
            nc.scalar.activation(
                std_sb[:], w_sb[:], AF.Sqrt, bias=eps_sb[0:64, :], scale=cnorm
            )
            rstd_sb = cp.tile([64, 1], F32)
            nc.vector.reciprocal(rstd_sb[:], std_sb[:])
            sb2 = cp.tile([64, 2], F32R)  # (scale, bias2) packed for PE rep
            nc.vector.tensor_mul(sb2[:, 0:1], rstd_sb[:], bnw_sb[0:64, :])
            u_sb = cp.tile([64, 1], F32)
            nc.vector.tensor_mul(u_sb[:], S1_ap, sb2[:, 0:1])
            nc.vector.tensor_scalar(
                sb2[:, 1:2], u_sb[:], -cnorm, bnb_sb[0:64, :], ALU.mult, ALU.add
            )
            # replicate (scale, bias2) to all 128 partitions via repW matmul
            with tc.tile_pool(name="psT", bufs=1, space="PSUM") as psT:
                rep_ps = psT.tile([128, 2], F32)
                nc.tensor.matmul(
                    rep_ps[:], repW_sb[:], sb2[:],
                    start=True, stop=True,
                )
                sb128 = cp.tile([128, 2], F32)
                nc.vector.tensor_copy(sb128[:], rep_ps[:])

            # R2 split layout [128, 1024]: R2[h*64+c, i'] = scale*G[...]+bias2.
            # Lower half on ACT (out = func(in*scale+bias)), upper half on DVE
            # tensor_scalar — the two engines run in parallel; 2 column chunks
            # each so the combine can start early.
            # G0-hi was pre-moved to partitions 64:128 under the AllGather.
            R2 = big.tile([128, 1024], BF16)
            r2_bounds = [0, 128, 384, 704, 1024]  # small first chunk so the
            for rc in range(4):                   # out-DMA stream starts early
                rsl = slice(r2_bounds[rc], r2_bounds[rc + 1])
                nc.scalar.activation(
                    R2[0:64, rsl], G0[:, rsl], AF.Identity,
                    scale=sb128[0:64, 0:1],
                    bias=sb128[0:64, 1:2],
                )
                nc.vector.tensor_scalar(
                    R2[64:128, rsl], G2hi[64:128, rsl], sb128[64:128, 0:1],
                    sb128[64:128, 1:2], ALU.mult, ALU.add,
                )

            # out2[p, f] = x2[p, f] + R2[p, f>>2]; 8 compute chunks split
            # 5 DVE / 3 Pool (Pool is ~2x slower per element), DMAs grouped
            # two chunks each to halve HWDGE serialization.
            out2 = big.tile([128, SX // 2], BF16)
            o_view = out2[:].rearrange("p (n u) -> p n u", u=4)
            x_view = x2[:].rearrange("p (n u) -> p n u", u=4)
            NFC = 8
            csz = (SX // 2) // NFC  # 512 output cols -> 128 R cols per chunk
            pool_chunks = {1, 4, 7}
            for qc in range(NFC):
                nsl = slice(qc * (csz // 4), (qc + 1) * (csz // 4))
                eng = nc.gpsimd if qc in pool_chunks else nc.vector
                eng.tensor_add(
                    o_view[:, nsl, :],
                    x_view[:, nsl, :],
                    _rep_ap(R2[:, nsl], 4),
                )
                nc.sync.dma_start(
                    out[:, qc * csz : (qc + 1) * csz],
                    out2[:, qc * csz : (qc + 1) * csz],
                )

    if split_waits:
        _split_excess_waits(nc)
    return nc


def _host_inputs(x, y, wq, bq, wk, bk, wv, bv, gamma, bn_w, bn_b):
    import ml_dtypes

    bf16 = ml_dtypes.bfloat16
    g = float(np.asarray(gamma).reshape(-1)[0])
    wqT_rep = np.tile(np.ascontiguousarray(wq.T), (1, 4))  # [256, 128]
    wkT_rep = np.tile(np.ascontiguousarray(wk.T), (1, 4))
    bv4g = (4.0 * g * bv)
    wpk = np.zeros((128, 704), np.float32)
    wpk[:, 0:128] = wqT_rep[0:128]
    wpk[:, 128:256] = wqT_rep[128:256]
    wpk[0, 256:384] = np.tile(bq, 4)
    wpk[:, 384:512] = wkT_rep[0:128]
    wpk[:, 512:640] = wkT_rep[128:256]
    # bk is dropped on-device: softmax over t is invariant to the key bias
    wpk[0:64, 640:704] = (g * wv).T
    wpk[64:128, 640:704] = (g * wv).T
    # repW: [64,128] with W[c,p]=1 iff p%64==c — PE partition replication
    wpkr = np.tile(np.eye(64, dtype=np.float32), (1, 2))
    msc = np.zeros((128, 8), np.float32)
    for hh in range(2):
        msc[hh * 64 : hh * 64 + 64, 0] = bv4g
        msc[hh * 64 : hh * 64 + 64, 1] = SY * bv4g
        msc[hh * 64 : hh * 64 + 64, 2] = 2.0 * bv4g
        msc[hh * 64 : hh * 64 + 64, 3] = SY * bv4g * bv4g
        msc[hh * 64 : hh * 64 + 64, 4] = bn_w
        msc[hh * 64 : hh * 64 + 64, 5] = bn_b
    common = {
        "wpk": wpk.astype(bf16),
        "wpkr": np.ascontiguousarray(wpkr),
        "msc": msc,
    }
    in_maps = []
    for b in range(B):
        m = dict(common)
        # split layout: [2, 64, 4096] where [h, c, f] = x[b, c, 4096h + f]
        xf = np.asarray(x[b], np.float32).reshape(64, 2, SX // 2).transpose(1, 0, 2)
        m["xb"] = np.ascontiguousarray(xf.reshape(128, SX // 2)).astype(bf16)
        m["yb"] = np.ascontiguousarray(
            np.asarray(y[b], np.float32).reshape(2, 128, SY)
        ).astype(bf16)
        in_maps.append(m)
    return in_maps


_NC_CACHE = {}


def kernel(x, y, wq, bq, wk, bk, wv, bv, gamma, bn_w, bn_b, _trace=False):
    from concourse.bass_utils import run_bass_kernel_spmd

    if "nc" not in _NC_CACHE:
        _NC_CACHE["nc"] = build_module()
    nc = _NC_CACHE["nc"]
    in_maps = _host_inputs(x, y, wq, bq, wk, bk, wv, bv, gamma, bn_w, bn_b)
    res = run_bass_kernel_spmd(
        nc, in_maps, core_ids=list(range(N_CORES)), trace=_trace
    )
    out = np.empty((B, CX, HX, WX), np.float32)
    for b in range(B):
        o2 = np.asarray(res.results[b]["out"]).astype(np.float32)
        o2 = o2.reshape(2, CX, SX // 2)
        out[b] = o2.transpose(1, 0, 2).reshape(CX, HX, WX)
    if _trace:
        _NC_CACHE["last_results"] = res
    return out



# revision 78
# speedup vs baseline: 1.0897x; 1.0381x over previous
"""Trainium2 Bass kernel for nn_Co_Pam_Module (PAM-style sparse attention +
nearest-upsample + BatchNorm residual).

Sharding: data-parallel over batch B=8 across 8 NeuronCores (one batch per
core); BN batch statistics are synchronized with a tiny AllReduce.

Math (validated vs reference, rel err ~1e-6 in numpy):
  q = wq@y + bq            [32, 2048]
  k = wk@y + bk            [32, 2048]
  E^T[t,s] = sum_d k[d,t] q[d,s]        (energy transposed; range ~±31 so
  P^T = exp(E^T)                         no max-subtraction is needed in f32)
  x_pool[c,j] = sum_u x[c,4j+u]
  vmm = (gamma*wv) @ x_pool             (gamma folded into weights)
  O~g[c,i] = sum_t vmm^T[t,c]*P^T[t,i]  via matmul with vpT=[vmm^T | ones];
  s[i]    = column 64 of the same accumulation (softmax denominator)
  G = O~g/s + 4*gamma*bv ; sync-BN stats via AllReduce of (sum G, sum G^2)
  out = x + scale_c*G_rep4 + bias_c
"""

import numpy as np

import concourse.bass as bass
import concourse.tile as tile
from concourse import library_config, mybir
from concourse.vector_clock import ScopedClock

F32 = mybir.dt.float32
F32R = mybir.dt.float32r
BF16 = mybir.dt.bfloat16
AF = mybir.ActivationFunctionType
ALU = mybir.AluOpType

# "rdma2" is the fast path in theory (3-round XOR gather-doubling over
# remote DMAs, ~4us vs the collective's 15us constant overhead) but this
# container's walrus cannot encode the SWDGE ISA structs (RemoteDMA*Descs,
# TriggerDma all fail codegen with "ISA wrong length"), so the collective
# is the only compilable cross-core sync.
SYNC_MODE = "collective"  # "rdma2" | "collective" | "none"

B, CX, HX, WX = 8, 64, 128, 64
CY, HY, WY = 256, 64, 32
SX, SY, D, RUP = HX * WX, HY * WY, 32, 4  # 8192, 2048, 32, 4
N_CORES = 8
BN_EPS = 1e-5
WPK_COLS = 968


# ---------------------------------------------------------------------------
# Workaround: walrus in this container rejects >cap sem waits on the Tile
# kernel-tail Drain.  Emit explicit per-sem wait_ge instructions instead.
def _patched_drain_and_barrier(self, tick_clock, wait_clock):
    nc = self.nc
    probe = nc.sync.nop(nofuse=True)
    wait_clock.add_sem_waits(probe.ins, ScopedClock({None: tick_clock.global_clock}))
    waits = list(probe.ins.sync_info.on_wait)
    probe.ins.sync_info.on_wait = []
    name2handle = {}
    for k, h in wait_clock.sems.allocated().items():
        name2handle[getattr(h, "name", str(k))] = h
    for w in waits:
        h = name2handle.get(w.ant_name)
        if h is None:
            raise RuntimeError(f"no sem handle for {w.ant_name}")
        nc.sync.wait_ge(h, w.wait_value)
    nc.sync.drain()
    nc.all_engine_barrier()
    popped = nc._tile_sem_poison_stack.pop()
    assert popped is self._sem_poison
    nc.clear_and_free_semaphores(list(self.sems.allocated().values()))
    nc.all_engine_barrier()


tile.TileContext._drain_and_barrier = _patched_drain_and_barrier


def _split_excess_waits(nc, cap=1):
    """Walrus in this container allows only `cap` sem waits per instruction.
    Hoist excess semaphore waits onto same-engine NoOps inserted just before
    the instruction (same engine + program order => semantics preserved)."""
    n_split = 0
    for f in nc.m.functions:
        for blk in f.blocks:
            insts = list(blk.instructions)
            new_insts = []
            changed = False
            for inst in insts:
                si = inst.sync_info
                waits = list(si.on_wait) if si is not None else []
                if len(waits) > cap:
                    sem_w = [w for w in waits if w.sync_type == "semaphore"]
                    other_w = [w for w in waits if w.sync_type != "semaphore"]
                    budget = max(0, cap - len(other_w))
                    keep, excess = sem_w[:budget], sem_w[budget:]
                    for i in range(0, len(excess), max(1, cap)):
                        chunk = excess[i : i + max(1, cap)]
                        nop = mybir.InstNoOp(
                            name=f"{inst.name}-ws{n_split}",
                            sync_info=mybir.SyncInfo(on_wait=chunk, on_update=[]),
                            bass_nofuse=True,
                            engine=inst.engine,
                        )
                        new_insts.append(nop)
                        n_split += 1
                    si.on_wait = other_w + keep
                    changed = True
                new_insts.append(inst)
            if changed:
                blk.instructions = new_insts
    return n_split
# ---------------------------------------------------------------------------


def _rep_ap(ap, rep):
    """Append a step-0 (repeat) innermost free dim to an AP."""
    return bass.AP(tensor=ap.tensor, offset=ap.offset, ap=list(ap.ap) + [[0, rep]])


def build_module(split_waits=True, sim_sync=False):
    nc = bass.Bass()

    # bf16 x/y inputs halve the 4.25MB input stream — every DMA arrival
    # (and thus exp0 and the k-quarter emissions) moves ~2x earlier.
    xb = nc.dram_tensor("xb", [128, SX // 2], BF16, kind="ExternalInput")
    yb = nc.dram_tensor("yb", [2, 128, SY], BF16, kind="ExternalInput")
    # packed weights: bf16 piece (wq/bq/wk/wv — matmul partners of bf16
    # data) + f32r piece (repW, partner of the f32r stats matmul)
    wpk = nc.dram_tensor("wpk", [128, 704], BF16, kind="ExternalInput")
    wpkr = nc.dram_tensor("wpkr", [64, 128], F32R, kind="ExternalInput")
    msc = nc.dram_tensor("msc", [128, 8], F32, kind="ExternalInput")
    # bf16 output halves the 2MB store stream (~2.9us saved on the tail);
    # host upcasts to f32.  out = x + BN(...) in bf16 costs ~2e-3 rel err
    # against a 2e-2 budget.
    out = nc.dram_tensor("out", [128, SX // 2], BF16, kind="ExternalOutput")

    with tile.TileContext(nc, num_cores=N_CORES) as tc:
        with (
            tc.tile_pool(name="const", bufs=1) as cp,
            tc.tile_pool(name="big", bufs=1) as big,
            tc.tile_pool(name="ptile", bufs=12) as pp,
            tc.tile_pool(name="dram", bufs=1, space="DRAM") as dp,
        ):
            # ---------------- constants / weights (three DMA pieces) ---------
            wpk_sb = cp.tile([128, 704], BF16)
            wpkr_sb = cp.tile([64, 128], F32R)
            # bf16 column map: 0:256 wqT(kc0,kc1), 256:384 bq row, 384:640
            # wkT, 640:704 wvT*gamma (stacked twice on partitions).  Pieces
            # sized so the q0 matmul chain unblocks as early as possible:
            # A=[0:384] (wq+bq), B=[384:640] (wk), C=[640:704]+wpkr.
            bq_sb = wpk_sb[0:1, 256:384]
            bk_sb = None  # key bias dropped (softmax invariance)
            wv_sb = wpk_sb[:, 640:704]
            repW_sb = wpkr_sb[:, :]
            nc.sync.dma_start(wpk_sb[:, 0:384], wpk[:, 0:384])
            msc_sb = cp.tile([128, 8], F32)
            bv4g_sb = msc_sb[0:64, 0:1]
            bv4g_sb2 = msc_sb[:, 0:1]  # [128,1]
            c_s1_sb = msc_sb[0:64, 1:2]
            bv4g2_sb = msc_sb[0:64, 2:3]
            c_s2_sb = msc_sb[0:64, 3:4]
            bnw_sb = msc_sb[:, 4:5]  # [128,1] both halves
            bnb_sb = msc_sb[:, 5:6]  # [128,1] both halves

            # prewarm the PE clock (HAM ramp): memset FIRST so the dummy
            # matmul chain starts as early as possible — full PE speed needs
            # 3us of continuous execution before the q/k/E matmuls
            pewarm = cp.tile([128, 512], F32R)
            nc.vector.memset(pewarm[:].bitcast(F32), 0.0)

            ones_row = cp.tile([1, 512], BF16)
            nc.vector.memset(ones_row[:], 1.0)
            ones64 = cp.tile([1, 64], F32R)
            nc.vector.memset(ones64[:].bitcast(F32), 1.0)
            eps_sb = cp.tile([128, 1], F32)
            nc.vector.memset(eps_sb[:], BN_EPS)

            # prewarm exp table early (overlaps initial DMA)
            warm = cp.tile([1, 8], F32)
            nc.vector.memset(warm[:], 0.0)
            nc.scalar.activation(warm[:], warm[:], AF.Exp)

            # ---------------- big inputs ----------------
            y_sb = big.tile([128, 2, SY], BF16)
            # x in split layout: partition h*64+c holds x[c, 4096h:4096(h+1)]
            x2 = big.tile([128, SX // 2], BF16)
            NXP = 4  # x pieces; pooling/vpT chunked to chase the DMA
            def y_quarter(ch):
                for kc in range(2):
                    nc.sync.dma_start(
                        y_sb[:, kc, ch * 512 : (ch + 1) * 512],
                        yb[kc][:, ch * 512 : (ch + 1) * 512],
                    )

            def x_piece(p, halves=False):
                if halves:
                    # finer pieces so pool_sub(0)'s chain (which gates O0 and
                    # the early E stream via PE program order) starts earlier
                    for q in range(2):
                        xsl = slice(p * 1024 + q * 512, p * 1024 + q * 512 + 512)
                        nc.sync.dma_start(x2[:, xsl], xb[:, xsl])
                else:
                    xsl = slice(p * 1024, (p + 1) * 1024)
                    nc.sync.dma_start(x2[:, xsl], xb[:, xsl])

            # ordered so each consumer's operand lands just before its first
            # use: wpk piece A precedes this block (q0 chain), piece B (wk)
            # lands before k0, piece C (wv) before pool_sub(0); x pieces
            # chase the vpT chain (iter 2p), late y quarters only gate E at
            # iters 8/12
            y_quarter(0)
            nc.sync.dma_start(wpk_sb[:, 384:640], wpk[:, 384:640])
            y_quarter(1)
            nc.sync.dma_start(wpk_sb[:, 640:], wpk[:, 640:])
            nc.sync.dma_start(wpkr_sb[:], wpkr[:])
            x_piece(0)
            x_piece(1)
            y_quarter(2)
            y_quarter(3)
            x_piece(2)
            x_piece(3)
            nc.sync.dma_start(msc_sb[:], msc[:])

            # sync-BN allreduce buffers + pre-generated rdma descriptors.
            # XOR gather-doubling: round r sends buf[:, 0:2^(r+1)] to peer
            # me^(2^r), landing at buf[:, 2^(r+1):2^(r+2)]; after 3 rounds all
            # 8 cores' [128,2] stats sit in buf's 8 slots.  Descriptor
            # generation (~1us each on Pool SEQ) is hoisted here so only the
            # trigger/flight/ack path remains on the post-loop critical path.
            # Recv slots (cols 2:16) are remote-written only — no local writes,
            # no init (cross-core W-after-W races otherwise).
            if SYNC_MODE == "rdma2":
                arbuf = cp.tile([128, 16], F32)
                arbuf2 = cp.tile([128, 16], F32)
                ar_rsems = [nc.alloc_semaphore(f"ar_r{r}") for r in range(3)]
                ar_lsem = nc.alloc_semaphore("ar_l")
                ar_psem = nc.alloc_semaphore("ar_prep")
                with tc.tile_critical():
                    g = nc.gpsimd
                    g.load_library(library_config.remote_dma)
                    for r in range(3):
                        delta = 1 << r
                        slot = 4 if (delta & 4) else 0
                        rd = [None] * 8
                        rd[slot] = (0, delta)
                        g.remote_dma_broadcast(
                            arbuf[:, 2 * (1 << r) : 2 * (1 << (r + 1))],
                            arbuf[:, 0 : 2 * (1 << r)],
                            remote_sem=ar_rsems[r],
                            local_sem=ar_lsem,
                            rdests=rd,
                        ).then_inc(ar_psem, 1)

            q_sb = big.tile([128, SY], F32R)
            k_sb = big.tile([128, SY], F32R)

            # ---------------- main compute: single PSUM regime ----------------
            # psE: 3 rotating [128,1024] slots (6 banks) shared by warmup/qk/
            # vpT/rs-broadcast/E tiles; psO: [65,1024] accumulator (2 banks).
            # h=1's O accumulation reuses the psO slot and therefore waits for
            # half-0's recip/bcast/mult reads (~2.9us); the 8-deep P pool lets
            # the E/exp pipeline run ahead so that stall is absorbed.
            t1 = big.tile([128, SX // 4], BF16)
            xp = big.tile([128, SX // 8], BF16)
            xv = x2[:].rearrange("p (n u) -> p n u", u=2)
            tv = t1[:].rearrange("p (n u) -> p n u", u=2)
            vpT = big.tile([128, 16, 65], F32R)
            nc.vector.memset(vpT[:, :, 64:65].bitcast(F32), 1.0)
            G0 = big.tile([64, SY], F32)
            s1_h = cp.tile([64, 2], F32)
            s2_h = cp.tile([64, 2], F32)
            junk2 = big.tile([64, 1024], F32, tag="junk2")
            junk3 = big.tile([64, 1024], F32, tag="junk3")
            with (
                tc.tile_pool(name="psE", bufs=3, space="PSUM") as psE,
                tc.tile_pool(name="psO", bufs=1, space="PSUM") as psO,
            ):
                # PE clock warmup — chain sized to bridge the gap until the
                # first q0 matmul's y data lands, so the ramp never resets and
                # the whole q/k/E chain runs at full clock
                wslot = psE.tile([128, 1024], F32, tag="E")
                for _ in range(7):
                    nc.tensor.matmul(
                        wslot[:, 0:512], pewarm[:, 0:128], pewarm[:],
                        start=True, stop=True,
                    )

                def emit_qk(w_off, b_t, dst, qt, with_bias=True):
                    # Key bias is dropped (with_bias=False for wk): softmax
                    # over t is invariant to the per-s column shift bk.q_s, so
                    # k = wk@y suffices — saves a 512-cycle PE matmul per
                    # quarter.
                    gslc = slice(qt * 512, (qt + 1) * 512)
                    ps = psE.tile([128, 1024], F32, tag="E")
                    for kc in range(2):
                        nc.tensor.matmul(
                            ps[:, 0:512],
                            wpk_sb[:, w_off + kc * 128 : w_off + kc * 128 + 128],
                            y_sb[:, kc, gslc],
                            start=(kc == 0),
                            stop=(kc == 1 and not with_bias),
                        )
                    if with_bias:
                        nc.tensor.matmul(
                            ps[:, 0:512], b_t[:], ones_row[:],
                            start=False, stop=True,
                        )
                    nc.vector.tensor_copy(dst[:, gslc], ps[:, 0:512])

                emit_qk(0, bq_sb, q_sb, 0)
                emit_qk(384, bk_sb, k_sb, 0, with_bias=False)
                emit_qk(0, bq_sb, q_sb, 1)

                def emit_pool_sub(jc):
                    # one 128-col xp window -> vpT chunks {2jc, 2jc+1}; spreads
                    # the piece work over two iterations to balance PE vs ACT
                    t1s = slice(jc * 256, (jc + 1) * 256)
                    nc.vector.tensor_add(t1[:, t1s], xv[:, t1s, 0], xv[:, t1s, 1])
                    xps = slice(jc * 128, (jc + 1) * 128)
                    nc.vector.tensor_add(xp[:, xps], tv[:, xps, 0], tv[:, xps, 1])
                    vps = psE.tile([128, 1024], F32, tag="E")
                    for hh in range(2):
                        base = slice(hh * 64, hh * 64 + 64)
                        # separate banks (cols 0 / 512): start=True bank clears
                        # cannot collide
                        nc.tensor.matmul(
                            vps[0:128, hh * 512 : hh * 512 + 64],
                            xp[base, xps], wv_sb[base, :],
                            start=True, stop=True,
                        )
                    vv = vps[:].rearrange("p (g c) -> p g c", c=512)
                    nc.vector.tensor_copy(vpT[:, 2 * jc : 2 * jc + 2, 0:64], vv[:, :, 0:64])

                def emit_half_tail(h, o_ps):
                    # walrus allows only ONE PSUM input per vector op, so rr
                    # bounces through SBUF.  h=0 runs hidden in-loop on DVE
                    # slack; h=1 is the critical post-loop path, pipelined in
                    # 2 column chunks across DVE (recip/mult/reduce), PE
                    # (broadcast) and ACT (rr copy + square accum).
                    isl_g = slice(h * 1024, (h + 1) * 1024)
                    rs_sb = big.tile([1, 1024], F32R, tag=f"rs{h}")
                    rr_ps = psE.tile([128, 1024], F32, tag="E")
                    rr_sb = big.tile([64, 1024], F32, tag=f"rsrep{h}")
                    if h == 0:
                        with nc.allow_low_precision(reason="fp32r softmax denom"):
                            nc.vector.reciprocal(rs_sb[:], o_ps[64:65, :])
                        # copy O (not rr) out of PSUM: releases the psO
                        # accumulator ~2us earlier so h=1's O matmuls unblock
                        o0_sb = big.tile([64, 1024], F32, tag="o0_sb")
                        nc.vector.tensor_copy(o0_sb[:], o_ps[0:64, :])
                        for c2 in range(2):
                            nc.tensor.matmul(
                                rr_ps[0:64, c2 * 512 : (c2 + 1) * 512],
                                ones64[:],
                                rs_sb[:, c2 * 512 : (c2 + 1) * 512],
                                start=True, stop=True,
                            )
                        nc.vector.tensor_mul(G0[:, isl_g], o0_sb[:], rr_ps[0:64, :])
                        nc.vector.tensor_reduce(
                            s1_h[:, h : h + 1], G0[:, isl_g],
                            mybir.AxisListType.X, ALU.add,
                        )
                    else:
                        with nc.allow_low_precision(reason="fp32r denom"):
                            nc.vector.reciprocal(rs_sb[:], o_ps[64:65, :])
                        for c2 in range(2):
                            nc.tensor.matmul(
                                rr_ps[0:64, c2 * 512 : (c2 + 1) * 512],
                                ones64[:],
                                rs_sb[:, c2 * 512 : (c2 + 1) * 512],
                                start=True, stop=True,
                            )
                        o_sb = big.tile([64, 1024], F32, tag="o1_sb")
                        nc.scalar.activation(o_sb[:], o_ps[0:64, :], AF.Copy)
                        nc.vector.tensor_mul(G0[:, isl_g], o_sb[:], rr_ps[0:64, :])
                        nc.vector.tensor_reduce(
                            s1_h[:, 1:2], G0[:, isl_g],
                            mybir.AxisListType.X, ALU.add,
                        )
                        nc.scalar.activation(
                            junk2[:], G0[:, isl_g], AF.Square,
                            accum_out=s2_h[:, 1:2],
                        )

                def emit_half_sq(h):
                    # half-0 squares on the otherwise-idle Pool engine,
                    # row-sum on DVE slack — fully hidden under the h=1
                    # loop (G0-lo is ready then); half-1's squares are fused
                    # into emit_half_tail(1)'s chunk pipeline.
                    assert h == 0
                    isl_g = slice(h * 1024, (h + 1) * 1024)
                    nc.gpsimd.tensor_mul(junk3[:], G0[:, isl_g], G0[:, isl_g])
                    nc.vector.tensor_reduce(
                        s2_h[:, 0:1], junk3[:], mybir.AxisListType.X, ALU.add
                    )

                def emit_E(h, ts):
                    tslc = slice(ts * 128, (ts + 1) * 128)
                    e_ps = psE.tile([128, 1024], F32, tag="E")
                    for c2 in range(2):
                        nc.tensor.matmul(
                            e_ps[:, c2 * 512 : (c2 + 1) * 512],
                            k_sb[0:32, tslc],
                            q_sb[0:32, h * 1024 + c2 * 512 : h * 1024 + c2 * 512 + 512],
                            start=True, stop=True,
                        )
                    return e_ps

                o_ps_prev = None
                # software-pipelined one iteration ahead: E(ts+1) issues on PE
                # before the (DMA-gated) pool/qk work and O(ts), so the exp
                # stream never waits behind them in PE program order
                e_cur = emit_E(0, 0)
                for h in range(2):
                    o_ps = psO.tile([65, 1024], F32, tag="O")
                    for ts in range(16):
                        p_sb = pp.tile([128, 1024], F32R, tag="P")
                        if h == 0 and ts == 0:
                            # first exp split in halves: chunk 0 only needs
                            # the q0 copy, so the ACT stream starts earlier
                            for c2 in range(2):
                                csl = slice(c2 * 512, (c2 + 1) * 512)
                                nc.scalar.activation(
                                    p_sb[:, csl], e_cur[:, csl], AF.Exp
                                )
                        else:
                            nc.scalar.activation(p_sb[:], e_cur[:], AF.Exp)
                        if ts < 15:
                            e_cur = emit_E(h, ts + 1)
                        elif h == 0:
                            e_cur = emit_E(1, 0)
                        if h == 0:
                            if ts < 8:
                                emit_pool_sub(ts)
                            if ts == 1:
                                emit_qk(384, bk_sb, k_sb, 1, with_bias=False)
                            elif ts == 5:
                                emit_qk(384, bk_sb, k_sb, 2, with_bias=False)
                            elif ts == 9:
                                emit_qk(384, bk_sb, k_sb, 3, with_bias=False)
                            elif ts == 10:
                                emit_qk(0, bq_sb, q_sb, 2)
                            elif ts == 12:
                                emit_qk(0, bq_sb, q_sb, 3)

                        for c2 in range(2):
                            nc.tensor.matmul(
                                o_ps[:, c2 * 512 : (c2 + 1) * 512],
                                vpT[:, 2 * (ts % 8) + ts // 8, :],
                                p_sb[:, c2 * 512 : (c2 + 1) * 512],
                                start=(ts == 0),
                                stop=(ts == 15),
                                skip_group_check=True,
                            )
                        if h == 1 and ts == 1 and o_ps_prev is not None:
                            emit_half_tail(0, o_ps_prev)
                        if h == 1 and ts == 6:
                            emit_half_sq(0)  # Pool engine, hidden in-loop
                    o_ps_prev = o_ps
                emit_half_tail(1, o_ps_prev)

            # ---------------- BN stats + AllReduce ----------------
            # Raw per-core sums only: s1 = sum_i G~[c,i], s2 = sum_i G~^2.
            # BN is invariant to the constant shift bv4g (G_true = G~ + bv4g):
            # var = a2 - a1^2 and bias2 = bn_b - scale*a1 with a1,a2 the
            # globally-averaged raw sums — no bias correction terms needed.
            ar_sb = cp.tile([64, 2], F32R)
            nc.vector.tensor_add(ar_sb[:, 0:1], s1_h[:, 0:1], s1_h[:, 1:2])
            nc.vector.tensor_add(ar_sb[:, 1:2], s2_h[:, 0:1], s2_h[:, 1:2])

            # pre-move G0-hi to partitions 64:128 (hidden under the AllReduce)
            G2hi = big.tile([128, 1024], F32, tag="g2hi")
            nc.sync.dma_start(G2hi[64:128, :], G0[:, 1024:2048])

            if SYNC_MODE == "rdma2":
                # replicate [64,2] stats to all 128 partitions via PE
                # (repW = [I64|I64] lives in wpk cols 840:968)
                with tc.tile_pool(name="psT", bufs=1, space="PSUM") as psT:
                    rep_ps = psT.tile([128, 2], F32)
                    nc.tensor.matmul(
                        rep_ps[:], repW_sb[:], ar_sb[:],
                        start=True, stop=True,
                    )
                    nc.vector.tensor_copy(arbuf[:, 0:2], rep_ps[:])
                # warm the Sqrt table while the allreduce is in flight
                rs_warm = cp.tile([64, 1], F32)
                nc.scalar.activation(rs_warm[:], s1_h[:, 0:1], AF.Sqrt)
                with tc.tile_critical():
                    g = nc.gpsimd
                    scr = cp.tile([128, 2], F32, tag="rdma_scr")
                    g.wait_ge(ar_psem, 3)
                    g.tensor_copy(scr[:], arbuf[:, 0:2])  # order trigger after stats
                    for r in range(3):
                        g.trigger_dma(count=1)
                        if sim_sync:
                            # TimelineSim cannot deliver peer sem updates;
                            # stand in for flight+ack latency then self-satisfy
                            g.tensor_copy(scr[:], arbuf[:, 0:2])
                            g.sem_inc(ar_rsems[r], 2)
                        g.wait_ge(ar_rsems[r], 2)
                    g.tensor_copy(arbuf2[:], arbuf[:])  # local sync point
                sums_tile = cp.tile([128, 2], F32)
                nc.vector.tensor_reduce(
                    sums_tile[:],
                    arbuf2[:].rearrange("p (s c) -> p c s", c=2),
                    mybir.AxisListType.X,
                    ALU.add,
                )
                sums_sb = sums_tile[:]
            elif SYNC_MODE == "collective":
                ar_in = dp.tile([64, 2], F32)
                ar_out = dp.tile([N_CORES, 64, 2], F32)
                nc.sync.dma_start(ar_in[:], ar_sb[:].bitcast(F32))
                nc.gpsimd.collective_compute(
                    "AllGather",
                    ALU.bypass,
                    ins=[ar_in.opt()],
                    outs=[ar_out.opt()],
                    replica_groups=[list(range(N_CORES))],
                )
                # single gather onto partitions 0:64 as [64, 2, 8], rank-reduce
                gath_sb = cp.tile([64, 2, N_CORES], F32)
                nc.sync.dma_start(
                    gath_sb[:], ar_out[:].rearrange("r c j -> c j r")
                )
                sums_tile = cp.tile([64, 2], F32)
                nc.vector.tensor_reduce(
                    sums_tile[:], gath_sb[:], mybir.AxisListType.X, ALU.add
                )
                sums_sb = sums_tile[:]
            else:
                # debug fallback: per-core stats scaled by B (exact only if all
                # batches had identical stats)
                sums_tile = cp.tile([64, 2], F32)
                bounce = dp.tile([64, 2], F32)
                nc.sync.dma_start(bounce[:], ar_sb[:].bitcast(F32))
                nc.sync.dma_start(sums_tile[:], bounce[:])
                nc.vector.tensor_scalar_mul(sums_tile[:], sums_tile[:], float(B))
                sums_sb = sums_tile[:]

            # ---------------- scale/bias (64-wide) + PE replication ----------
            # From raw allreduced sums S1,S2 with c = 1/(B*SY):
            #   var = c*(S2 - c*S1^2);  scale = bn_w/sqrt(var+eps);
            #   bias2 = bn_b - scale*c*S1
            # (BN is invariant to the bv4g shift, so no bias corrections.)
            cnorm = float(RUP) / (B * SX)
            S1_ap = sums_sb[:, 0:1]
            S2_ap = sums_sb[:, 1:2]
            m2_sb = cp.tile([64, 1], F32)
            nc.vector.tensor_mul(m2_sb[:], S1_ap, S1_ap)
            w_sb = cp.tile([64, 1], F32)  # S2 - c*S1^2
            nc.vector.tensor_scalar(
                w_sb[:], m2_sb[:], -cnorm, S2_ap, ALU.mult, ALU.add
            )
            std_sb = cp.tile([64, 1], F32)
            nc.scalar.activation(
                std_sb[:], w_sb[:], AF.Sqrt, bias=eps_sb[0:64, :], scale=cnorm
            )
            rstd_sb = cp.tile([64, 1], F32)
            nc.vector.reciprocal(rstd_sb[:], std_sb[:])
            sb2 = cp.tile([64, 2], F32R)  # (scale, bias2) packed for PE rep
            nc.vector.tensor_mul(sb2[:, 0:1], rstd_sb[:], bnw_sb[0:64, :])
            u_sb = cp.tile([64, 1], F32)
            nc.vector.tensor_mul(u_sb[:], S1_ap, sb2[:, 0:1])
            nc.vector.tensor_scalar(
                sb2[:, 1:2], u_sb[:], -cnorm, bnb_sb[0:64, :], ALU.mult, ALU.add
            )
            # replicate (scale, bias2) to all 128 partitions via repW matmul
            with tc.tile_pool(name="psT", bufs=1, space="PSUM") as psT:
                rep_ps = psT.tile([128, 2], F32)
                nc.tensor.matmul(
                    rep_ps[:], repW_sb[:], sb2[:],
                    start=True, stop=True,
                )
                sb128 = cp.tile([128, 2], F32)
                nc.vector.tensor_copy(sb128[:], rep_ps[:])

            # R2 split layout [128, 1024]: R2[h*64+c, i'] = scale*G[...]+bias2.
            # Lower half on ACT (out = func(in*scale+bias)), upper half on DVE
            # tensor_scalar — the two engines run in parallel; 2 column chunks
            # each so the combine can start early.
            # G0-hi was pre-moved to partitions 64:128 under the AllGather.
            R2 = big.tile([128, 1024], BF16)
            r2_bounds = [0, 128, 384, 704, 1024]  # small first chunk so the
            for rc in range(4):                   # out-DMA stream starts early
                rsl = slice(r2_bounds[rc], r2_bounds[rc + 1])
                nc.scalar.activation(
                    R2[0:64, rsl], G0[:, rsl], AF.Identity,
                    scale=sb128[0:64, 0:1],
                    bias=sb128[0:64, 1:2],
                )
                nc.vector.tensor_scalar(
                    R2[64:128, rsl], G2hi[64:128, rsl], sb128[64:128, 0:1],
                    sb128[64:128, 1:2], ALU.mult, ALU.add,
                )

            # out2[p, f] = x2[p, f] + R2[p, f>>2]; 8 compute chunks split
            # 5 DVE / 3 Pool (Pool is ~2x slower per element), DMAs grouped
            # two chunks each to halve HWDGE serialization.
            out2 = big.tile([128, SX // 2], BF16)
            o_view = out2[:].rearrange("p (n u) -> p n u", u=4)
            x_view = x2[:].rearrange("p (n u) -> p n u", u=4)
            NFC = 8
            csz = (SX // 2) // NFC  # 512 output cols -> 128 R cols per chunk
            pool_chunks = {1, 4, 7}
            for qc in range(NFC):
                nsl = slice(qc * (csz // 4), (qc + 1) * (csz // 4))
                eng = nc.gpsimd if qc in pool_chunks else nc.vector
                eng.tensor_add(
                    o_view[:, nsl, :],
                    x_view[:, nsl, :],
                    _rep_ap(R2[:, nsl], 4),
                )
                # bf16 transfers are 364ns vs HWDGE's 625ns/instruction —
                # group two compute chunks per DMA so HWDGE isn't the limiter
                if qc % 2 == 1:
                    nc.sync.dma_start(
                        out[:, (qc - 1) * csz : (qc + 1) * csz],
                        out2[:, (qc - 1) * csz : (qc + 1) * csz],
                    )

    if split_waits:
        _split_excess_waits(nc)
    return nc


def _host_inputs(x, y, wq, bq, wk, bk, wv, bv, gamma, bn_w, bn_b):
    import ml_dtypes

    bf16 = ml_dtypes.bfloat16
    g = float(np.asarray(gamma).reshape(-1)[0])
    wqT_rep = np.tile(np.ascontiguousarray(wq.T), (1, 4))  # [256, 128]
    wkT_rep = np.tile(np.ascontiguousarray(wk.T), (1, 4))
    bv4g = (4.0 * g * bv)
    wpk = np.zeros((128, 704), np.float32)
    wpk[:, 0:128] = wqT_rep[0:128]
    wpk[:, 128:256] = wqT_rep[128:256]
    wpk[0, 256:384] = np.tile(bq, 4)
    wpk[:, 384:512] = wkT_rep[0:128]
    wpk[:, 512:640] = wkT_rep[128:256]
    # bk is dropped on-device: softmax over t is invariant to the key bias
    wpk[0:64, 640:704] = (g * wv).T
    wpk[64:128, 640:704] = (g * wv).T
    # repW: [64,128] with W[c,p]=1 iff p%64==c — PE partition replication
    wpkr = np.tile(np.eye(64, dtype=np.float32), (1, 2))
    msc = np.zeros((128, 8), np.float32)
    for hh in range(2):
        msc[hh * 64 : hh * 64 + 64, 0] = bv4g
        msc[hh * 64 : hh * 64 + 64, 1] = SY * bv4g
        msc[hh * 64 : hh * 64 + 64, 2] = 2.0 * bv4g
        msc[hh * 64 : hh * 64 + 64, 3] = SY * bv4g * bv4g
        msc[hh * 64 : hh * 64 + 64, 4] = bn_w
        msc[hh * 64 : hh * 64 + 64, 5] = bn_b
    common = {
        "wpk": wpk.astype(bf16),
        "wpkr": np.ascontiguousarray(wpkr),
        "msc": msc,
    }
    in_maps = []
    for b in range(B):
        m = dict(common)
        # split layout: [2, 64, 4096] where [h, c, f] = x[b, c, 4096h + f]
        xf = np.asarray(x[b], np.float32).reshape(64, 2, SX // 2).transpose(1, 0, 2)
        m["xb"] = np.ascontiguousarray(xf.reshape(128, SX // 2)).astype(bf16)
        m["yb"] = np.ascontiguousarray(
            np.asarray(y[b], np.float32).reshape(2, 128, SY)
        ).astype(bf16)
        in_maps.append(m)
    return in_maps


_NC_CACHE = {}


def kernel(x, y, wq, bq, wk, bk, wv, bv, gamma, bn_w, bn_b, _trace=False):
    from concourse.bass_utils import run_bass_kernel_spmd

    if "nc" not in _NC_CACHE:
        _NC_CACHE["nc"] = build_module()
    nc = _NC_CACHE["nc"]
    in_maps = _host_inputs(x, y, wq, bq, wk, bk, wv, bv, gamma, bn_w, bn_b)
    res = run_bass_kernel_spmd(
        nc, in_maps, core_ids=list(range(N_CORES)), trace=_trace
    )
    out = np.empty((B, CX, HX, WX), np.float32)
    for b in range(B):
        o2 = np.asarray(res.results[b]["out"]).astype(np.float32)
        o2 = o2.reshape(2, CX, SX // 2)
        out[b] = o2.transpose(1, 0, 2).reshape(CX, HX, WX)
    if _trace:
        _NC_CACHE["last_results"] = res
    return out



# revision 81
# speedup vs baseline: 1.1139x; 1.0222x over previous
"""Trainium2 Bass kernel for nn_Co_Pam_Module (PAM-style sparse attention +
nearest-upsample + BatchNorm residual).

Sharding: data-parallel over batch B=8 across 8 NeuronCores (one batch per
core); BN batch statistics are synchronized with a tiny AllReduce.

Math (validated vs reference, rel err ~1e-6 in numpy):
  q = wq@y + bq            [32, 2048]
  k = wk@y + bk            [32, 2048]
  E^T[t,s] = sum_d k[d,t] q[d,s]        (energy transposed; range ~±31 so
  P^T = exp(E^T)                         no max-subtraction is needed in f32)
  x_pool[c,j] = sum_u x[c,4j+u]
  vmm = (gamma*wv) @ x_pool             (gamma folded into weights)
  O~g[c,i] = sum_t vmm^T[t,c]*P^T[t,i]  via matmul with vpT=[vmm^T | ones];
  s[i]    = column 64 of the same accumulation (softmax denominator)
  G = O~g/s + 4*gamma*bv ; sync-BN stats via AllReduce of (sum G, sum G^2)
  out = x + scale_c*G_rep4 + bias_c
"""

import numpy as np

import concourse.bass as bass
import concourse.tile as tile
from concourse import library_config, mybir
from concourse.vector_clock import ScopedClock

F32 = mybir.dt.float32
F32R = mybir.dt.float32r
BF16 = mybir.dt.bfloat16
AF = mybir.ActivationFunctionType
ALU = mybir.AluOpType

# "rdma2" is the fast path in theory (3-round XOR gather-doubling over
# remote DMAs, ~4us vs the collective's 15us constant overhead) but this
# container's walrus cannot encode the SWDGE ISA structs (RemoteDMA*Descs,
# TriggerDma all fail codegen with "ISA wrong length"), so the collective
# is the only compilable cross-core sync.
SYNC_MODE = "collective"  # "rdma2" | "collective" | "none"

B, CX, HX, WX = 8, 64, 128, 64
CY, HY, WY = 256, 64, 32
SX, SY, D, RUP = HX * WX, HY * WY, 32, 4  # 8192, 2048, 32, 4
N_CORES = 8
BN_EPS = 1e-5
WPK_COLS = 968


# ---------------------------------------------------------------------------
# Workaround: walrus in this container rejects >cap sem waits on the Tile
# kernel-tail Drain.  Emit explicit per-sem wait_ge instructions instead.
def _patched_drain_and_barrier(self, tick_clock, wait_clock):
    nc = self.nc
    probe = nc.sync.nop(nofuse=True)
    wait_clock.add_sem_waits(probe.ins, ScopedClock({None: tick_clock.global_clock}))
    waits = list(probe.ins.sync_info.on_wait)
    probe.ins.sync_info.on_wait = []
    name2handle = {}
    for k, h in wait_clock.sems.allocated().items():
        name2handle[getattr(h, "name", str(k))] = h
    for w in waits:
        h = name2handle.get(w.ant_name)
        if h is None:
            raise RuntimeError(f"no sem handle for {w.ant_name}")
        nc.sync.wait_ge(h, w.wait_value)
    nc.sync.drain()
    nc.all_engine_barrier()
    popped = nc._tile_sem_poison_stack.pop()
    assert popped is self._sem_poison
    nc.clear_and_free_semaphores(list(self.sems.allocated().values()))
    nc.all_engine_barrier()


tile.TileContext._drain_and_barrier = _patched_drain_and_barrier


def _split_excess_waits(nc, cap=1):
    """Walrus in this container allows only `cap` sem waits per instruction.
    Hoist excess semaphore waits onto same-engine NoOps inserted just before
    the instruction (same engine + program order => semantics preserved)."""
    n_split = 0
    for f in nc.m.functions:
        for blk in f.blocks:
            insts = list(blk.instructions)
            new_insts = []
            changed = False
            for inst in insts:
                si = inst.sync_info
                waits = list(si.on_wait) if si is not None else []
                if len(waits) > cap:
                    sem_w = [w for w in waits if w.sync_type == "semaphore"]
                    other_w = [w for w in waits if w.sync_type != "semaphore"]
                    budget = max(0, cap - len(other_w))
                    keep, excess = sem_w[:budget], sem_w[budget:]
                    for i in range(0, len(excess), max(1, cap)):
                        chunk = excess[i : i + max(1, cap)]
                        nop = mybir.InstNoOp(
                            name=f"{inst.name}-ws{n_split}",
                            sync_info=mybir.SyncInfo(on_wait=chunk, on_update=[]),
                            bass_nofuse=True,
                            engine=inst.engine,
                        )
                        new_insts.append(nop)
                        n_split += 1
                    si.on_wait = other_w + keep
                    changed = True
                new_insts.append(inst)
            if changed:
                blk.instructions = new_insts
    return n_split
# ---------------------------------------------------------------------------


def _rep_ap(ap, rep):
    """Append a step-0 (repeat) innermost free dim to an AP."""
    return bass.AP(tensor=ap.tensor, offset=ap.offset, ap=list(ap.ap) + [[0, rep]])


def build_module(split_waits=True, sim_sync=False):
    nc = bass.Bass()

    # bf16 x/y inputs halve the 4.25MB input stream — every DMA arrival
    # (and thus exp0 and the k-quarter emissions) moves ~2x earlier.
    xb = nc.dram_tensor("xb", [128, SX // 2], BF16, kind="ExternalInput")
    yb = nc.dram_tensor("yb", [2, 128, SY], BF16, kind="ExternalInput")
    # packed weights: bf16 piece (wq/bq/wk/wv — matmul partners of bf16
    # data) + f32r piece (repW, partner of the f32r stats matmul)
    wpk = nc.dram_tensor("wpk", [128, 704], BF16, kind="ExternalInput")
    wpkr = nc.dram_tensor("wpkr", [64, 128], F32R, kind="ExternalInput")
    msc = nc.dram_tensor("msc", [128, 8], F32, kind="ExternalInput")
    # bf16 output halves the 2MB store stream (~2.9us saved on the tail);
    # host upcasts to f32.  out = x + BN(...) in bf16 costs ~2e-3 rel err
    # against a 2e-2 budget.
    out = nc.dram_tensor("out", [128, SX // 2], BF16, kind="ExternalOutput")

    with tile.TileContext(nc, num_cores=N_CORES) as tc:
        with (
            tc.tile_pool(name="const", bufs=1) as cp,
            tc.tile_pool(name="big", bufs=1) as big,
            tc.tile_pool(name="ptile", bufs=12) as pp,
            tc.tile_pool(name="dram", bufs=1, space="DRAM") as dp,
        ):
            # ---------------- constants / weights (three DMA pieces) ---------
            wpk_sb = cp.tile([128, 704], BF16)
            wpkr_sb = cp.tile([64, 128], F32R)
            # bf16 column map: 0:256 wqT(kc0,kc1), 256:384 bq row, 384:640
            # wkT, 640:704 wvT*gamma (stacked twice on partitions).  Pieces
            # sized so the q0 matmul chain unblocks as early as possible:
            # A=[0:384] (wq+bq), B=[384:640] (wk), C=[640:704]+wpkr.
            bq_sb = wpk_sb[0:1, 256:384]
            bk_sb = None  # key bias dropped (softmax invariance)
            wv_sb = wpk_sb[:, 640:704]
            repW_sb = wpkr_sb[:, :]
            nc.sync.dma_start(wpk_sb[:, 0:384], wpk[:, 0:384])
            msc_sb = cp.tile([128, 8], F32)
            bv4g_sb = msc_sb[0:64, 0:1]
            bv4g_sb2 = msc_sb[:, 0:1]  # [128,1]
            c_s1_sb = msc_sb[0:64, 1:2]
            bv4g2_sb = msc_sb[0:64, 2:3]
            c_s2_sb = msc_sb[0:64, 3:4]
            bnw_sb = msc_sb[:, 4:5]  # [128,1] both halves
            bnb_sb = msc_sb[:, 5:6]  # [128,1] both halves

            # prewarm the PE clock (HAM ramp): memset FIRST so the dummy
            # matmul chain starts as early as possible — full PE speed needs
            # 3us of continuous execution before the q/k/E matmuls
            pewarm = cp.tile([128, 512], F32R)
            nc.vector.memset(pewarm[:].bitcast(F32), 0.0)

            ones_row = cp.tile([1, 512], BF16)
            nc.vector.memset(ones_row[:], 1.0)
            ones64 = cp.tile([1, 64], F32R)
            nc.vector.memset(ones64[:].bitcast(F32), 1.0)
            eps_sb = cp.tile([128, 1], F32)
            nc.vector.memset(eps_sb[:], BN_EPS)

            # prewarm exp table early (overlaps initial DMA)
            warm = cp.tile([1, 8], F32)
            nc.vector.memset(warm[:], 0.0)
            nc.scalar.activation(warm[:], warm[:], AF.Exp)

            # ---------------- big inputs ----------------
            y_sb = big.tile([128, 2, SY], BF16)
            # x in split layout: partition h*64+c holds x[c, 4096h:4096(h+1)]
            x2 = big.tile([128, SX // 2], BF16)
            NXP = 4  # x pieces; pooling/vpT chunked to chase the DMA
            def y_quarter(ch):
                for kc in range(2):
                    nc.sync.dma_start(
                        y_sb[:, kc, ch * 512 : (ch + 1) * 512],
                        yb[kc][:, ch * 512 : (ch + 1) * 512],
                    )

            def x_piece(p, halves=False):
                if halves:
                    # finer pieces so pool_sub(0)'s chain (which gates O0 and
                    # the early E stream via PE program order) starts earlier
                    for q in range(2):
                        xsl = slice(p * 1024 + q * 512, p * 1024 + q * 512 + 512)
                        nc.sync.dma_start(x2[:, xsl], xb[:, xsl])
                else:
                    xsl = slice(p * 1024, (p + 1) * 1024)
                    nc.sync.dma_start(x2[:, xsl], xb[:, xsl])

            # ordered so each consumer's operand lands just before its first
            # use: wpk piece A precedes this block (q0 chain), piece B (wk)
            # lands before k0, piece C (wv) before pool_sub(0); x pieces
            # chase the vpT chain (iter 2p), late y quarters only gate E at
            # iters 8/12
            y_quarter(0)
            nc.sync.dma_start(wpk_sb[:, 384:640], wpk[:, 384:640])
            y_quarter(1)
            nc.sync.dma_start(wpk_sb[:, 640:], wpk[:, 640:])
            nc.sync.dma_start(wpkr_sb[:], wpkr[:])
            x_piece(0)
            x_piece(1)
            y_quarter(2)
            y_quarter(3)
            x_piece(2)
            x_piece(3)
            nc.sync.dma_start(msc_sb[:], msc[:])

            # sync-BN allreduce buffers + pre-generated rdma descriptors.
            # XOR gather-doubling: round r sends buf[:, 0:2^(r+1)] to peer
            # me^(2^r), landing at buf[:, 2^(r+1):2^(r+2)]; after 3 rounds all
            # 8 cores' [128,2] stats sit in buf's 8 slots.  Descriptor
            # generation (~1us each on Pool SEQ) is hoisted here so only the
            # trigger/flight/ack path remains on the post-loop critical path.
            # Recv slots (cols 2:16) are remote-written only — no local writes,
            # no init (cross-core W-after-W races otherwise).
            if SYNC_MODE == "rdma2":
                arbuf = cp.tile([128, 16], F32)
                arbuf2 = cp.tile([128, 16], F32)
                ar_rsems = [nc.alloc_semaphore(f"ar_r{r}") for r in range(3)]
                ar_lsem = nc.alloc_semaphore("ar_l")
                ar_psem = nc.alloc_semaphore("ar_prep")
                with tc.tile_critical():
                    g = nc.gpsimd
                    g.load_library(library_config.remote_dma)
                    for r in range(3):
                        delta = 1 << r
                        slot = 4 if (delta & 4) else 0
                        rd = [None] * 8
                        rd[slot] = (0, delta)
                        g.remote_dma_broadcast(
                            arbuf[:, 2 * (1 << r) : 2 * (1 << (r + 1))],
                            arbuf[:, 0 : 2 * (1 << r)],
                            remote_sem=ar_rsems[r],
                            local_sem=ar_lsem,
                            rdests=rd,
                        ).then_inc(ar_psem, 1)

            q_sb = big.tile([128, SY], F32R)
            k_sb = big.tile([128, SY], F32R)

            # ---------------- main compute: single PSUM regime ----------------
            # psE: 3 rotating [128,1024] slots (6 banks) shared by warmup/qk/
            # vpT/rs-broadcast/E tiles; psO: [65,1024] accumulator (2 banks).
            # h=1's O accumulation reuses the psO slot and therefore waits for
            # half-0's recip/bcast/mult reads (~2.9us); the 8-deep P pool lets
            # the E/exp pipeline run ahead so that stall is absorbed.
            t1 = big.tile([128, SX // 4], BF16)
            xp = big.tile([128, SX // 8], BF16)
            xv = x2[:].rearrange("p (n u) -> p n u", u=2)
            tv = t1[:].rearrange("p (n u) -> p n u", u=2)
            vpT = big.tile([128, 16, 65], F32R)
            nc.vector.memset(vpT[:, :, 64:65].bitcast(F32), 1.0)
            G0 = big.tile([64, SY], F32)
            s1_h = cp.tile([64, 2], F32)
            s2_h = cp.tile([64, 2], F32)
            junk2 = big.tile([64, 1024], F32, tag="junk2")
            junk3 = big.tile([64, 1024], F32, tag="junk3")
            with (
                tc.tile_pool(name="psE", bufs=3, space="PSUM") as psE,
                tc.tile_pool(name="psO", bufs=1, space="PSUM") as psO,
            ):
                # PE clock warmup — chain sized to bridge the gap until the
                # first q0 matmul's y data lands, so the ramp never resets and
                # the whole q/k/E chain runs at full clock
                wslot = psE.tile([128, 1024], F32, tag="E")
                for _ in range(7):
                    nc.tensor.matmul(
                        wslot[:, 0:512], pewarm[:, 0:128], pewarm[:],
                        start=True, stop=True,
                    )

                def emit_qk(w_off, b_t, dst, qt, with_bias=True):
                    # Key bias is dropped (with_bias=False for wk): softmax
                    # over t is invariant to the per-s column shift bk.q_s, so
                    # k = wk@y suffices — saves a 512-cycle PE matmul per
                    # quarter.
                    gslc = slice(qt * 512, (qt + 1) * 512)
                    ps = psE.tile([128, 1024], F32, tag="E")
                    for kc in range(2):
                        nc.tensor.matmul(
                            ps[:, 0:512],
                            wpk_sb[:, w_off + kc * 128 : w_off + kc * 128 + 128],
                            y_sb[:, kc, gslc],
                            start=(kc == 0),
                            stop=(kc == 1 and not with_bias),
                        )
                    if with_bias:
                        nc.tensor.matmul(
                            ps[:, 0:512], b_t[:], ones_row[:],
                            start=False, stop=True,
                        )
                    nc.vector.tensor_copy(dst[:, gslc], ps[:, 0:512])

                emit_qk(0, bq_sb, q_sb, 0)
                emit_qk(384, bk_sb, k_sb, 0, with_bias=False)
                emit_qk(0, bq_sb, q_sb, 1)

                def emit_pool_sub(jc):
                    # one 128-col xp window -> vpT chunks {2jc, 2jc+1}; spreads
                    # the piece work over two iterations to balance PE vs ACT
                    t1s = slice(jc * 256, (jc + 1) * 256)
                    nc.vector.tensor_add(t1[:, t1s], xv[:, t1s, 0], xv[:, t1s, 1])
                    xps = slice(jc * 128, (jc + 1) * 128)
                    nc.vector.tensor_add(xp[:, xps], tv[:, xps, 0], tv[:, xps, 1])
                    vps = psE.tile([128, 1024], F32, tag="E")
                    for hh in range(2):
                        base = slice(hh * 64, hh * 64 + 64)
                        # separate banks (cols 0 / 512): start=True bank clears
                        # cannot collide
                        nc.tensor.matmul(
                            vps[0:128, hh * 512 : hh * 512 + 64],
                            xp[base, xps], wv_sb[base, :],
                            start=True, stop=True,
                        )
                    vv = vps[:].rearrange("p (g c) -> p g c", c=512)
                    nc.vector.tensor_copy(vpT[:, 2 * jc : 2 * jc + 2, 0:64], vv[:, :, 0:64])

                def emit_half_tail(h, o_ps):
                    # walrus allows only ONE PSUM input per vector op, so rr
                    # bounces through SBUF.  h=0 runs hidden in-loop on DVE
                    # slack; h=1 is the critical post-loop path, pipelined in
                    # 2 column chunks across DVE (recip/mult/reduce), PE
                    # (broadcast) and ACT (rr copy + square accum).
                    isl_g = slice(h * 1024, (h + 1) * 1024)
                    rs_sb = big.tile([1, 1024], F32R, tag=f"rs{h}")
                    rr_ps = psE.tile([128, 1024], F32, tag="E")
                    rr_sb = big.tile([64, 1024], F32, tag=f"rsrep{h}")
                    if h == 0:
                        with nc.allow_low_precision(reason="fp32r softmax denom"):
                            nc.vector.reciprocal(rs_sb[:], o_ps[64:65, :])
                        # copy O (not rr) out of PSUM: releases the psO
                        # accumulator ~2us earlier so h=1's O matmuls unblock
                        o0_sb = big.tile([64, 1024], F32, tag="o0_sb")
                        nc.vector.tensor_copy(o0_sb[:], o_ps[0:64, :])
                        for c2 in range(2):
                            nc.tensor.matmul(
                                rr_ps[0:64, c2 * 512 : (c2 + 1) * 512],
                                ones64[:],
                                rs_sb[:, c2 * 512 : (c2 + 1) * 512],
                                start=True, stop=True,
                            )
                        nc.vector.tensor_mul(G0[:, isl_g], o0_sb[:], rr_ps[0:64, :])
                        nc.vector.tensor_reduce(
                            s1_h[:, h : h + 1], G0[:, isl_g],
                            mybir.AxisListType.X, ALU.add,
                        )
                    else:
                        with nc.allow_low_precision(reason="fp32r denom"):
                            nc.vector.reciprocal(rs_sb[:], o_ps[64:65, :])
                        for c2 in range(2):
                            nc.tensor.matmul(
                                rr_ps[0:64, c2 * 512 : (c2 + 1) * 512],
                                ones64[:],
                                rs_sb[:, c2 * 512 : (c2 + 1) * 512],
                                start=True, stop=True,
                            )
                        # o-copy (ACT) and mult (DVE) pipelined in halves;
                        # recip and reduce stay monolithic (chunking those
                        # regressed — per-op overhead exceeds the overlap)
                        o_sb = big.tile([64, 1024], F32, tag="o1_sb")
                        s2c = cp.tile([64, 2], F32, tag="s2c")
                        for c2 in range(2):
                            csl = slice(c2 * 512, (c2 + 1) * 512)
                            gsl = slice(1024 + c2 * 512, 1024 + c2 * 512 + 512)
                            nc.scalar.activation(
                                o_sb[:, csl], o_ps[0:64, csl], AF.Copy
                            )
                            nc.vector.tensor_mul(
                                G0[:, gsl], o_sb[:, csl], rr_ps[0:64, csl]
                            )
                            nc.scalar.activation(
                                junk2[:, csl], G0[:, gsl], AF.Square,
                                accum_out=s2c[:, c2 : c2 + 1],
                            )
                        nc.vector.tensor_reduce(
                            s1_h[:, 1:2], G0[:, isl_g],
                            mybir.AxisListType.X, ALU.add,
                        )
                        nc.vector.tensor_add(
                            s2_h[:, 1:2], s2c[:, 0:1], s2c[:, 1:2]
                        )

                def emit_half_sq(h):
                    # half-0 squares on the otherwise-idle Pool engine,
                    # row-sum on DVE slack — fully hidden under the h=1
                    # loop (G0-lo is ready then); half-1's squares are fused
                    # into emit_half_tail(1)'s chunk pipeline.
                    assert h == 0
                    isl_g = slice(h * 1024, (h + 1) * 1024)
                    nc.gpsimd.tensor_mul(junk3[:], G0[:, isl_g], G0[:, isl_g])
                    nc.vector.tensor_reduce(
                        s2_h[:, 0:1], junk3[:], mybir.AxisListType.X, ALU.add
                    )

                def emit_E(h, ts):
                    tslc = slice(ts * 128, (ts + 1) * 128)
                    e_ps = psE.tile([128, 1024], F32, tag="E")
                    for c2 in range(2):
                        nc.tensor.matmul(
                            e_ps[:, c2 * 512 : (c2 + 1) * 512],
                            k_sb[0:32, tslc],
                            q_sb[0:32, h * 1024 + c2 * 512 : h * 1024 + c2 * 512 + 512],
                            start=True, stop=True,
                        )
                    return e_ps

                o_ps_prev = None
                # software-pipelined one iteration ahead: E(ts+1) issues on PE
                # before the (DMA-gated) pool/qk work and O(ts), so the exp
                # stream never waits behind them in PE program order
                e_cur = emit_E(0, 0)
                for h in range(2):
                    o_ps = psO.tile([65, 1024], F32, tag="O")
                    for ts in range(16):
                        p_sb = pp.tile([128, 1024], F32R, tag="P")
                        if h == 0 and ts == 0:
                            # first exp split in halves: chunk 0 only needs
                            # the q0 copy, so the ACT stream starts earlier
                            for c2 in range(2):
                                csl = slice(c2 * 512, (c2 + 1) * 512)
                                nc.scalar.activation(
                                    p_sb[:, csl], e_cur[:, csl], AF.Exp
                                )
                        else:
                            nc.scalar.activation(p_sb[:], e_cur[:], AF.Exp)
                        if ts < 15:
                            e_cur = emit_E(h, ts + 1)
                        elif h == 0:
                            e_cur = emit_E(1, 0)
                        def emit_O(ots, op_sb):
                            for c2 in range(2):
                                nc.tensor.matmul(
                                    o_ps[:, c2 * 512 : (c2 + 1) * 512],
                                    vpT[:, 2 * (ots % 8) + ots // 8, :],
                                    op_sb[:, c2 * 512 : (c2 + 1) * 512],
                                    start=(ots == 0),
                                    stop=(ots == 15),
                                    skip_group_check=True,
                                )

                        if h == 0:
                            if ts < 8:
                                emit_pool_sub(ts)
                            if ts == 1:
                                emit_qk(384, bk_sb, k_sb, 1, with_bias=False)
                            elif ts == 5:
                                emit_qk(384, bk_sb, k_sb, 2, with_bias=False)
                            elif ts == 9:
                                emit_qk(384, bk_sb, k_sb, 3, with_bias=False)
                            elif ts == 10:
                                emit_qk(0, bq_sb, q_sb, 2)
                            elif ts == 12:
                                emit_qk(0, bq_sb, q_sb, 3)

                        # O deferred one iteration so the x/vpT-gated chain
                        # never sits ahead of the next E in PE program order
                        if ts > 0:
                            emit_O(ts - 1, p_prev)
                        if ts == 15:
                            emit_O(15, p_sb)
                        p_prev = p_sb
                        if h == 1 and ts == 1 and o_ps_prev is not None:
                            emit_half_tail(0, o_ps_prev)
                        if h == 1 and ts == 6:
                            emit_half_sq(0)  # Pool engine, hidden in-loop
                    o_ps_prev = o_ps
                emit_half_tail(1, o_ps_prev)

            # ---------------- BN stats + AllReduce ----------------
            # Raw per-core sums only: s1 = sum_i G~[c,i], s2 = sum_i G~^2.
            # BN is invariant to the constant shift bv4g (G_true = G~ + bv4g):
            # var = a2 - a1^2 and bias2 = bn_b - scale*a1 with a1,a2 the
            # globally-averaged raw sums — no bias correction terms needed.
            ar_sb = cp.tile([64, 2], F32R)
            nc.vector.tensor_add(ar_sb[:, 0:1], s1_h[:, 0:1], s1_h[:, 1:2])
            nc.vector.tensor_add(ar_sb[:, 1:2], s2_h[:, 0:1], s2_h[:, 1:2])

            # pre-move G0-hi to partitions 64:128 (hidden under the AllReduce)
            G2hi = big.tile([128, 1024], F32, tag="g2hi")
            nc.sync.dma_start(G2hi[64:128, :], G0[:, 1024:2048])

            if SYNC_MODE == "rdma2":
                # replicate [64,2] stats to all 128 partitions via PE
                # (repW = [I64|I64] lives in wpk cols 840:968)
                with tc.tile_pool(name="psT", bufs=1, space="PSUM") as psT:
                    rep_ps = psT.tile([128, 2], F32)
                    nc.tensor.matmul(
                        rep_ps[:], repW_sb[:], ar_sb[:],
                        start=True, stop=True,
                    )
                    nc.vector.tensor_copy(arbuf[:, 0:2], rep_ps[:])
                # warm the Sqrt table while the allreduce is in flight
                rs_warm = cp.tile([64, 1], F32)
                nc.scalar.activation(rs_warm[:], s1_h[:, 0:1], AF.Sqrt)
                with tc.tile_critical():
                    g = nc.gpsimd
                    scr = cp.tile([128, 2], F32, tag="rdma_scr")
                    g.wait_ge(ar_psem, 3)
                    g.tensor_copy(scr[:], arbuf[:, 0:2])  # order trigger after stats
                    for r in range(3):
                        g.trigger_dma(count=1)
                        if sim_sync:
                            # TimelineSim cannot deliver peer sem updates;
                            # stand in for flight+ack latency then self-satisfy
                            g.tensor_copy(scr[:], arbuf[:, 0:2])
                            g.sem_inc(ar_rsems[r], 2)
                        g.wait_ge(ar_rsems[r], 2)
                    g.tensor_copy(arbuf2[:], arbuf[:])  # local sync point
                sums_tile = cp.tile([128, 2], F32)
                nc.vector.tensor_reduce(
                    sums_tile[:],
                    arbuf2[:].rearrange("p (s c) -> p c s", c=2),
                    mybir.AxisListType.X,
                    ALU.add,
                )
                sums_sb = sums_tile[:]
            elif SYNC_MODE == "collective":
                ar_in = dp.tile([64, 2], F32)
                ar_out = dp.tile([N_CORES, 64, 2], F32)
                nc.sync.dma_start(ar_in[:], ar_sb[:].bitcast(F32))
                nc.gpsimd.collective_compute(
                    "AllGather",
                    ALU.bypass,
                    ins=[ar_in.opt()],
                    outs=[ar_out.opt()],
                    replica_groups=[list(range(N_CORES))],
                )
                # single gather onto partitions 0:64 as [64, 2, 8], rank-reduce
                gath_sb = cp.tile([64, 2, N_CORES], F32)
                nc.sync.dma_start(
                    gath_sb[:], ar_out[:].rearrange("r c j -> c j r")
                )
                sums_tile = cp.tile([64, 2], F32)
                nc.vector.tensor_reduce(
                    sums_tile[:], gath_sb[:], mybir.AxisListType.X, ALU.add
                )
                sums_sb = sums_tile[:]
            else:
                # debug fallback: per-core stats scaled by B (exact only if all
                # batches had identical stats)
                sums_tile = cp.tile([64, 2], F32)
                bounce = dp.tile([64, 2], F32)
                nc.sync.dma_start(bounce[:], ar_sb[:].bitcast(F32))
                nc.sync.dma_start(sums_tile[:], bounce[:])
                nc.vector.tensor_scalar_mul(sums_tile[:], sums_tile[:], float(B))
                sums_sb = sums_tile[:]

            # ---------------- scale/bias (64-wide) + PE replication ----------
            # From raw allreduced sums S1,S2 with c = 1/(B*SY):
            #   var = c*(S2 - c*S1^2);  scale = bn_w/sqrt(var+eps);
            #   bias2 = bn_b - scale*c*S1
            # (BN is invariant to the bv4g shift, so no bias corrections.)
            cnorm = float(RUP) / (B * SX)
            S1_ap = sums_sb[:, 0:1]
            S2_ap = sums_sb[:, 1:2]
            m2_sb = cp.tile([64, 1], F32)
            nc.vector.tensor_mul(m2_sb[:], S1_ap, S1_ap)
            w_sb = cp.tile([64, 1], F32)  # S2 - c*S1^2
            nc.vector.tensor_scalar(
                w_sb[:], m2_sb[:], -cnorm, S2_ap, ALU.mult, ALU.add
            )
            std_sb = cp.tile([64, 1], F32)
            nc.scalar.activation(
                std_sb[:], w_sb[:], AF.Sqrt, bias=eps_sb[0:64, :], scale=cnorm
            )
            rstd_sb = cp.tile([64, 1], F32)
            nc.vector.reciprocal(rstd_sb[:], std_sb[:])
            sb2 = cp.tile([64, 2], F32R)  # (scale, bias2) packed for PE rep
            nc.vector.tensor_mul(sb2[:, 0:1], rstd_sb[:], bnw_sb[0:64, :])
            u_sb = cp.tile([64, 1], F32)
            nc.vector.tensor_mul(u_sb[:], S1_ap, sb2[:, 0:1])
            nc.vector.tensor_scalar(
                sb2[:, 1:2], u_sb[:], -cnorm, bnb_sb[0:64, :], ALU.mult, ALU.add
            )
            # replicate (scale, bias2) to all 128 partitions via repW matmul
            with tc.tile_pool(name="psT", bufs=1, space="PSUM") as psT:
                rep_ps = psT.tile([128, 2], F32)
                nc.tensor.matmul(
                    rep_ps[:], repW_sb[:], sb2[:],
                    start=True, stop=True,
                )
                sb128 = cp.tile([128, 2], F32)
                nc.vector.tensor_copy(sb128[:], rep_ps[:])

            # R2 split layout [128, 1024]: R2[h*64+c, i'] = scale*G[...]+bias2.
            # Lower half on ACT (out = func(in*scale+bias)), upper half on DVE
            # tensor_scalar — the two engines run in parallel; 2 column chunks
            # each so the combine can start early.
            # G0-hi was pre-moved to partitions 64:128 under the AllGather.
            R2 = big.tile([128, 1024], BF16)
            r2_bounds = [0, 128, 384, 704, 1024]  # small first chunk so the
            for rc in range(4):                   # out-DMA stream starts early
                rsl = slice(r2_bounds[rc], r2_bounds[rc + 1])
                nc.scalar.activation(
                    R2[0:64, rsl], G0[:, rsl], AF.Identity,
                    scale=sb128[0:64, 0:1],
                    bias=sb128[0:64, 1:2],
                )
                nc.vector.tensor_scalar(
                    R2[64:128, rsl], G2hi[64:128, rsl], sb128[64:128, 0:1],
                    sb128[64:128, 1:2], ALU.mult, ALU.add,
                )

            # out2[p, f] = x2[p, f] + R2[p, f>>2]; 8 compute chunks split
            # 5 DVE / 3 Pool (Pool is ~2x slower per element), DMAs grouped
            # two chunks each to halve HWDGE serialization.
            out2 = big.tile([128, SX // 2], BF16)
            o_view = out2[:].rearrange("p (n u) -> p n u", u=4)
            x_view = x2[:].rearrange("p (n u) -> p n u", u=4)
            NFC = 8
            csz = (SX // 2) // NFC  # 512 output cols -> 128 R cols per chunk
            pool_chunks = {1, 4, 7}
            for qc in range(NFC):
                nsl = slice(qc * (csz // 4), (qc + 1) * (csz // 4))
                eng = nc.gpsimd if qc in pool_chunks else nc.vector
                eng.tensor_add(
                    o_view[:, nsl, :],
                    x_view[:, nsl, :],
                    _rep_ap(R2[:, nsl], 4),
                )
                # bf16 transfers are 364ns vs HWDGE's 625ns/instruction —
                # group two compute chunks per DMA so HWDGE isn't the limiter
                if qc % 2 == 1:
                    nc.sync.dma_start(
                        out[:, (qc - 1) * csz : (qc + 1) * csz],
                        out2[:, (qc - 1) * csz : (qc + 1) * csz],
                    )

    if split_waits:
        _split_excess_waits(nc)
    return nc


def _host_inputs(x, y, wq, bq, wk, bk, wv, bv, gamma, bn_w, bn_b):
    import ml_dtypes

    bf16 = ml_dtypes.bfloat16
    g = float(np.asarray(gamma).reshape(-1)[0])
    wqT_rep = np.tile(np.ascontiguousarray(wq.T), (1, 4))  # [256, 128]
    wkT_rep = np.tile(np.ascontiguousarray(wk.T), (1, 4))
    bv4g = (4.0 * g * bv)
    wpk = np.zeros((128, 704), np.float32)
    wpk[:, 0:128] = wqT_rep[0:128]
    wpk[:, 128:256] = wqT_rep[128:256]
    wpk[0, 256:384] = np.tile(bq, 4)
    wpk[:, 384:512] = wkT_rep[0:128]
    wpk[:, 512:640] = wkT_rep[128:256]
    # bk is dropped on-device: softmax over t is invariant to the key bias
    wpk[0:64, 640:704] = (g * wv).T
    wpk[64:128, 640:704] = (g * wv).T
    # repW: [64,128] with W[c,p]=1 iff p%64==c — PE partition replication
    wpkr = np.tile(np.eye(64, dtype=np.float32), (1, 2))
    msc = np.zeros((128, 8), np.float32)
    for hh in range(2):
        msc[hh * 64 : hh * 64 + 64, 0] = bv4g
        msc[hh * 64 : hh * 64 + 64, 1] = SY * bv4g
        msc[hh * 64 : hh * 64 + 64, 2] = 2.0 * bv4g
        msc[hh * 64 : hh * 64 + 64, 3] = SY * bv4g * bv4g
        msc[hh * 64 : hh * 64 + 64, 4] = bn_w
        msc[hh * 64 : hh * 64 + 64, 5] = bn_b
    common = {
        "wpk": wpk.astype(bf16),
        "wpkr": np.ascontiguousarray(wpkr),
        "msc": msc,
    }
    in_maps = []
    for b in range(B):
        m = dict(common)
        # split layout: [2, 64, 4096] where [h, c, f] = x[b, c, 4096h + f]
        xf = np.asarray(x[b], np.float32).reshape(64, 2, SX // 2).transpose(1, 0, 2)
        m["xb"] = np.ascontiguousarray(xf.reshape(128, SX // 2)).astype(bf16)
        m["yb"] = np.ascontiguousarray(
            np.asarray(y[b], np.float32).reshape(2, 128, SY)
        ).astype(bf16)
        in_maps.append(m)
    return in_maps


_NC_CACHE = {}


def kernel(x, y, wq, bq, wk, bk, wv, bv, gamma, bn_w, bn_b, _trace=False):
    from concourse.bass_utils import run_bass_kernel_spmd

    if "nc" not in _NC_CACHE:
        _NC_CACHE["nc"] = build_module()
    nc = _NC_CACHE["nc"]
    in_maps = _host_inputs(x, y, wq, bq, wk, bk, wv, bv, gamma, bn_w, bn_b)
    res = run_bass_kernel_spmd(
        nc, in_maps, core_ids=list(range(N_CORES)), trace=_trace
    )
    out = np.empty((B, CX, HX, WX), np.float32)
    for b in range(B):
        o2 = np.asarray(res.results[b]["out"]).astype(np.float32)
        o2 = o2.reshape(2, CX, SX // 2)
        out[b] = o2.transpose(1, 0, 2).reshape(CX, HX, WX)
    if _trace:
        _NC_CACHE["last_results"] = res
    return out

